# revision 32
# baseline (speedup 1.0000x reference)
"""MHSA (dense transformer, ALiBi + causal) TRN2 Bass kernel, 8-core SPMD.

v4 design:
- Sharding: batch (2) x head-quartile -> 8 cores, heads REBALANCED so every
  core gets one head from each ALiBi-slope quartile: core c (b=c//4, k=c%4)
  computes heads [12+k, 8+k, 4+k, 0+k] (0-indexed) of batch b. Slot s on all
  cores then shares one causal band -> SPMD-friendly block skipping.
- Banded causal attention: ALiBi slope*(i-j) >~ 32 => weight < e^-32,
  skipped structurally. Bands (in 128-blocks) per slot: [16, 16, 5, 2].
- All matmuls bf16 (1 cyc/row on PE at any width). ALiBi folded into 3
  bf16-exact aug contraction rows:
    Q~ = [Q; slope; slope; -slope*i],  K~ = [K; j_hi; j_lo; 1]
  with j_hi multiple of 256 and j_lo in [0,256) - both exact in bf16; the
  -slope*i row is a per-row shift that cancels in softmax.
- Projection phase (x/w bf16, PSUM f32): Q^T/K^T copied to bf16 SBUF slot
  tiles by DVE, V by DVE into a [j, slot, 65] bf16 tile with a ones column
  (column 64 of the AV output becomes the softmax denominator).
- Attention interleaved with projections by i-window: window t = i in
  [512t, 512t+512), its strip groups round-robined with the projection
  chains of superblock t+1 so ACT exp overlaps PE projection matmuls.
- Scores S^T[j,i] per (slot, J): strips grouped into <=1024-col PSUM tiles,
  ONE exp (ACT) per group -> bf16 P^T; diagonal 128-strips masked on Pool
  (gpsimd) with a 0/1 triangle.
- AV: out[i(128), 65] accumulated over J into a [128, 4, 80] PSUM tile.
  IMPORTANT: PSUM accumulation groups are tracked per 2KB bank - only ONE
  chain may be open per bank at a time (interleaved starts lazily re-zero
  the bank). So all AV chains of a window are emitted at window END, one
  i-block at a time, each chain fully closed before the next starts.
- Epilogue = batched reciprocal + tensor_scalar multiplies (DVE) into an
  SBUF staging tile, DMA'd out per slot-pair (512B elements).
"""

import numpy as np
import ml_dtypes

import concourse.bass as bass
import concourse.mybir as mybir
import concourse.tile as tile
from concourse import bacc
from concourse.bass_utils import run_bass_kernel_spmd

P = 128
S = 2048
D = 1024
H = 16
HWID = 64
HPC = 4           # head slots per core
CW = HPC * HWID   # 256
NKC = D // P      # 8 contraction chunks
NTSB = 4          # t super-blocks (projection + attention i-windows)
TSB = S // NTSB   # 512
NJ = S // P       # 16
AUG = 3
QROWS = HWID + AUG  # 67
VSTR = 72           # V~ sub-block stride: 64 data + aligned ones lane
VONE = 68           # ones column (softmax denominator) within the lane
BANDS = [16, 16, 5, 2]      # causal band per slot, in 128-blocks
SLOT_BASE = [12, 8, 4, 0]   # head (0-indexed) = SLOT_BASE[s] + (core % 4)

F32 = mybir.dt.float32
BF16 = mybir.dt.bfloat16

EXP_BIAS = -12.0
BF16NP = ml_dtypes.bfloat16


def window_strips(s, t):
    """Strips (J, i0, i1) of window t for slot s (banded causal)."""
    B = BANDS[s]
    res = []
    for J in range(max(0, 4 * t - B + 1), 4 * t + 4):
        i0 = max(TSB * t, P * J)
        i1 = min(TSB * t + TSB, P * (J + B), S)
        if i1 > i0:
            res.append((J, i0, i1))
    return res


def group_strips(strips_w, cap=1024):
    groups, cur, w = [], [], 0
    for (J, i0, i1) in strips_w:
        if w + (i1 - i0) > cap and cur:
            groups.append(cur)
            cur, w = [], 0
        cur.append((J, i0, i1))
        w += i1 - i0
    if cur:
        groups.append(cur)
    return groups


def build_kernel():
    nc = bacc.Bacc("TRN2")

    xq = nc.dram_tensor("xq", [D, S], BF16, kind="ExternalInput")
    xk = nc.dram_tensor("xk", [D, S], BF16, kind="ExternalInput")
    xv = nc.dram_tensor("xv", [D, S], BF16, kind="ExternalInput")
    wq = nc.dram_tensor("wq", [D, CW], BF16, kind="ExternalInput")
    wk = nc.dram_tensor("wk", [D, CW], BF16, kind="ExternalInput")
    wv = nc.dram_tensor("wv", [D, CW], BF16, kind="ExternalInput")
    augq = nc.dram_tensor("augq", [HPC, AUG, S], BF16, kind="ExternalInput")
    augk = nc.dram_tensor("augk", [AUG, S], BF16, kind="ExternalInput")
    tri = nc.dram_tensor("tri", [P, P], BF16, kind="ExternalInput")
    out = nc.dram_tensor("out", [S, CW], F32, kind="ExternalOutput")

    x_drams = [xq, xk, xv]
    w_drams = [wq, wk, wv]

    with tile.TileContext(nc) as tc:
        with (
            tc.tile_pool(name="cn", bufs=1) as cnp,
            tc.tile_pool(name="wp", bufs=1) as wp,
            tc.tile_pool(name="qk", bufs=1) as qkp,
            tc.tile_pool(name="vv", bufs=1) as vvp,
            tc.tile_pool(name="xp", bufs=2) as xp,
            tc.tile_pool(name="pt", bufs=12) as ptp,
            tc.tile_pool(name="rc", bufs=3) as rcp,
            tc.tile_pool(name="ob", bufs=1) as obp,
            tc.tile_pool(name="pq", bufs=3, space="PSUM") as pqp,
            tc.tile_pool(name="sc", bufs=2, space="PSUM") as scp,
            tc.tile_pool(name="av", bufs=1, space="PSUM") as avp,
        ):
            # ---- startup DMAs in critical-path order: (wv, xv0), (wq, xq0),
            # slot-0/1 augs, (wk, xk0), slot-2/3 augs — V/Q chains and the
            # first windows (slots 0/1) start as soon as their data lands ----
            def emit_xdma_one(xt, t, xi):
                nc.sync.dma_start(
                    xt[:, :, xi, :],
                    x_drams[xi].rearrange("(ko p) t -> p ko t", p=P)[
                        :, :, t * TSB:(t + 1) * TSB],
                )

            def emit_xdma(t):
                xt = xp.tile([P, NKC, 3, TSB], BF16, tag="x", name=f"x{t}")
                for xi in range(3):
                    emit_xdma_one(xt, t, xi)
                return xt

            w_tiles = [None, None, None]
            xt0 = xp.tile([P, NKC, 3, TSB], BF16, tag="x", name="x0")

            def emit_wdma(i):
                wt = wp.tile([P, NKC, CW], BF16, tag=f"w{i}", name=f"w{i}")
                nc.sync.dma_start(
                    wt[:], w_drams[i].rearrange("(ko p) c -> p ko c", p=P))
                w_tiles[i] = wt

            def emit_xdma_half(xt, t, xi, h):
                # half-column DMA so the first chains start at half-transfer
                nc.sync.dma_start(
                    xt[:, :, xi, h * (TSB // 2):(h + 1) * (TSB // 2)],
                    x_drams[xi].rearrange("(ko p) t -> p ko t", p=P)[
                        :, :, t * TSB + h * (TSB // 2):
                        t * TSB + (h + 1) * (TSB // 2)],
                )

            emit_wdma(2)
            emit_xdma_half(xt0, 0, 2, 0)
            emit_xdma_half(xt0, 0, 2, 1)
            emit_wdma(0)
            emit_xdma_half(xt0, 0, 0, 0)
            emit_xdma_half(xt0, 0, 0, 1)

            # ---- constants ----
            ebias = cnp.tile([P, 1], F32, tag="ebias", name="ebias")
            nc.gpsimd.memset(ebias[:], EXP_BIAS)
            # preload the Exp activation table off the critical path
            actwarm = cnp.tile([P, 1], F32, tag="actwarm", name="actwarm")
            nc.scalar.activation(actwarm[:], ebias[:],
                                 mybir.ActivationFunctionType.Exp)

            # ---- per-slot Q~^T / K~^T [67, S] bf16; V~ [128, NJ, HPC, 65] ----
            qs = [qkp.tile([QROWS, S], BF16, tag=f"q{s}", name=f"q{s}")
                  for s in range(HPC)]
            ks = [qkp.tile([QROWS, S], BF16, tag=f"k{s}", name=f"k{s}")
                  for s in range(HPC)]
            # V~ sub-block = 72 cols: V data 0:64, then a 16-byte aligned
            # constant lane [64:72) holding the softmax-denominator ones at
            # col VONE. Padding keeps any >=4-byte RMW write granule away
            # from the bf16 V data (same-engine memsets, ordered before the
            # DVE V copies).
            v_t = vvp.tile([P, NJ, HPC, VSTR], BF16, tag="v", name="v_t")
            nc.vector.memset(v_t[:, :, :, HWID:VSTR], 0.0)
            nc.vector.memset(v_t[:, :, :, VONE:VONE + 1], 1.0)

            # then (wk, xk0); x1 chunks interleaved with the aug/tri consts
            # in demand order (xq1 before augs: t1 Q-chains fire first;
            # xv1 last: t1 V-chains run at the end of the w0 block)
            emit_wdma(1)
            emit_xdma_one(xt0, 0, 1)
            xt1 = xp.tile([P, NKC, 3, TSB], BF16, tag="x", name="x1")
            emit_xdma_one(xt1, 1, 0)
            for s in range(2):
                nc.sync.dma_start(qs[s][HWID:QROWS, :], augq[s])
                nc.sync.dma_start(ks[s][HWID:QROWS, :], augk[:])
            emit_xdma_one(xt1, 1, 1)
            tri_t = cnp.tile([P, P], BF16, tag="tri", name="tri_t")
            nc.sync.dma_start(tri_t[:], tri[:])
            for s in range(2, HPC):
                nc.sync.dma_start(qs[s][HWID:QROWS, :], augq[s])
                nc.sync.dma_start(ks[s][HWID:QROWS, :], augk[:])
            emit_xdma_one(xt1, 1, 2)

            # ---- output staging [P, NJ, CW] f32 ----
            out_sb = obp.tile([P, NJ, CW], F32, tag="ob", name="out_sb")

            # ---------- emission helpers ----------
            def proj_chain_units(t, xt):
                units = []
                if t == 0:
                    # V first (xv lands first), then Q/K cc0 (slots 0/1 ->
                    # first windows), then cc1
                    for u in range(TSB // P):
                        units.append(("v", t, xt, u))
                    for cc in range(2):
                        for pi in range(2):
                            units.append(("qk", t, xt, pi, cc))
                    return units
                for pi in range(2):
                    for cc in range(2):
                        units.append(("qk", t, xt, pi, cc))
                for u in range(TSB // P):
                    units.append(("v", t, xt, u))
                return units

            def emit_chain(unit):
                kind = unit[0]
                if kind == "qk":
                    _, t, xt, pi, cc = unit
                    dsts = qs if pi == 0 else ks
                    ps = pqp.tile([P, TSB], F32, tag="pq",
                                  name=f"pq{t}_{pi}_{cc}")
                    # t=0 cc0: two sequential half-chains so the first one
                    # starts after the half x-DMA lands
                    halves = ([(0, TSB // 2), (TSB // 2, TSB)]
                              if (t == 0 and cc == 0) else [(0, TSB)])
                    for (a, b) in halves:
                        for kk in range(NKC):
                            nc.tensor.matmul(
                                ps[:, a:b],
                                lhsT=w_tiles[pi][:, kk, cc * P:(cc + 1) * P],
                                rhs=xt[:, kk, pi, a:b],
                                start=(kk == 0),
                                stop=(kk == NKC - 1),
                            )
                    nc.vector.tensor_copy(
                        dsts[2 * cc][0:HWID, t * TSB:(t + 1) * TSB],
                        ps[0:HWID, :],
                    )
                    nc.vector.tensor_copy(
                        dsts[2 * cc + 1][0:HWID, t * TSB:(t + 1) * TSB],
                        ps[HWID:P, :],
                    )
                else:
                    _, t, xt, u = unit
                    tt = t * (TSB // P) + u
                    ps = pqp.tile([P, TSB], F32, tag="pq", name=f"pv{t}_{u}")
                    for kk in range(NKC):
                        nc.tensor.matmul(
                            ps[:, 0:CW],
                            lhsT=xt[:, kk, 2, u * P:(u + 1) * P],
                            rhs=w_tiles[2][:, kk, :],
                            start=(kk == 0),
                            stop=(kk == NKC - 1),
                        )
                    nc.vector.tensor_copy(
                        v_t[:, tt, :, 0:HWID],
                        ps[:, 0:CW].rearrange("p (h w) -> p h w", h=HPC),
                    )

            def emit_scores_exp(s, t, g, gi):
                width = sum(i1 - i0 for (_, i0, i1) in g)
                sc = scp.tile([P, 1024], F32, tag="sc", name=f"sc{s}_{t}_{gi}")
                o = 0
                offs = []
                for (J, i0, i1) in g:
                    W = i1 - i0
                    a = 0
                    while a < W:  # split at psum bank boundaries (512 cols)
                        b = min(W, a + 512 - (o + a) % 512)
                        nc.tensor.matmul(
                            sc[:, o + a:o + b],
                            lhsT=ks[s][0:QROWS, P * J:P * (J + 1)],
                            rhs=qs[s][0:QROWS, i0 + a:i0 + b],
                            start=True,
                            stop=True,
                        )
                        a = b
                    offs.append((J, i0, i1, o))
                    o += W
                pt = ptp.tile([P, 1024], BF16, tag="pt", name=f"pt{s}_{t}_{gi}")
                nc.scalar.activation(
                    pt[:, 0:width], sc[:, 0:width],
                    mybir.ActivationFunctionType.Exp,
                    bias=ebias[:], scale=1.0,
                )
                for (J, i0, i1, off) in offs:
                    if i0 == P * J:
                        # mask the diagonal 128-strip (keep j <= i), on Pool
                        nc.gpsimd.tensor_mul(
                            pt[:, off:off + P], pt[:, off:off + P], tri_t[:]
                        )
                return pt, offs

            def emit_epilogue(s, t, acc):
                rec = rcp.tile([P, 4, 1], F32, tag="rc", name=f"rc{s}_{t}")
                nc.vector.reciprocal(rec[:], acc[:, :, VONE:VONE + 1])
                for r in range(4):
                    nc.vector.tensor_scalar_mul(
                        out_sb[:, 4 * t + r, s * HWID:(s + 1) * HWID],
                        acc[:, r, 0:HWID],
                        rec[:, r, :],
                    )
                if t == NTSB - 1 and s % 2 == 1:
                    # slot pair done: 128 contiguous f32 columns -> 512B elems
                    nc.sync.dma_start(
                        out.rearrange("(a p) c -> p a c", p=P)[
                            :, :, (s - 1) * HWID:(s + 1) * HWID],
                        out_sb[:, :, (s - 1) * HWID:(s + 1) * HWID],
                    )

            # pending completed window: (s, t, acc, [(pt, offs), ...])
            pend_w = None
            pend_age = 0  # groups emitted since pend_w was set

            def flush_window():
                nonlocal pend_w
                if pend_w is None:
                    return
                s, t, acc, recs = pend_w
                B = BANDS[s]
                jmap = {}
                for pt, offs in recs:
                    for (J, i0, i1, off) in offs:
                        jmap[J] = (pt, off, i0)
                # one fully-closed accumulation chain per i-block (PSUM bank
                # allows only one open chain at a time)
                for r in range(4):
                    i_blk = 4 * t + r
                    jst = max(0, i_blk - B + 1)
                    for J in range(jst, i_blk + 1):
                        pt, off, i0 = jmap[J]
                        col = off + i_blk * P - i0
                        nc.tensor.matmul(
                            acc[:, r, 0:VONE + 1],
                            lhsT=pt[:, col:col + P],
                            rhs=v_t[:, J, s, 0:VONE + 1],
                            start=(J == jst),
                            stop=(J == i_blk),
                        )
                emit_epilogue(s, t, acc)
                pend_w = None

            # ---------- main schedule ----------
            for unit in proj_chain_units(0, xt0):
                emit_chain(unit)

            for t in range(NTSB):
                if t + 1 < NTSB:
                    xt_next = xt1 if t == 0 else emit_xdma(t + 1)
                    next_chains = proj_chain_units(t + 1, xt_next)
                else:
                    next_chains = []
                gitems = []
                for s in range(HPC):
                    gitems.append((s, group_strips(window_strips(s, t))))
                total_groups = sum(len(g) for _, g in gitems)
                ci = 0
                gcount = 0
                for s, groups in gitems:
                    acc = avp.tile([P, 4, 80], F32, tag="acc",
                                   name=f"acc{s}_{t}")
                    recs = []
                    for gi, g in enumerate(groups):
                        pt, offs = emit_scores_exp(s, t, g, gi)
                        recs.append((pt, offs))
                        gcount += 1
                        pend_age += 1
                        while (ci < len(next_chains)
                               and ci < (gcount * len(next_chains))
                               // total_groups):
                            emit_chain(next_chains[ci])
                            ci += 1
                        if pend_age >= 3:
                            flush_window()
                    flush_window()  # ensure previous window drained
                    pend_w = (s, t, acc, recs)
                    pend_age = 0
                while ci < len(next_chains):
                    emit_chain(next_chains[ci])
                    ci += 1
            flush_window()

    nc.compile()
    return nc


_NC = None


def _get_nc():
    global _NC
    if _NC is None:
        _NC = build_kernel()
    return _NC


def kernel(queries, keys, values, mask, Wq, Wk, Wv):
    B = queries.shape[0]
    asc = np.ascontiguousarray
    scale = 1.0 / np.sqrt(HWID)

    WqT = asc((np.asarray(Wq).T * scale).astype(np.float32)).astype(BF16NP)
    WkT = asc(np.asarray(Wk).T.astype(np.float32)).astype(BF16NP)
    WvT = asc(np.asarray(Wv).T.astype(np.float32)).astype(BF16NP)
    qTs = [asc(np.asarray(queries[b]).T.astype(np.float32)).astype(BF16NP)
           for b in range(B)]
    kTs = [asc(np.asarray(keys[b]).T.astype(np.float32)).astype(BF16NP)
           for b in range(B)]
    vTs = [asc(np.asarray(values[b]).T.astype(np.float32)).astype(BF16NP)
           for b in range(B)]

    slopes = (2.0 ** (-np.arange(1, H + 1) * (8.0 / H))).astype(np.float32)
    slopes_bf = slopes.astype(BF16NP).astype(np.float32)
    iv = np.arange(S, dtype=np.float32)
    j_hi = (np.arange(S) // 256 * 256).astype(np.float32)
    j_lo = (np.arange(S) % 256).astype(np.float32)
    augk_np = np.stack([j_hi, j_lo, np.ones(S, np.float32)]).astype(BF16NP)
    tri_np = np.asarray(
        np.arange(P)[:, None] <= np.arange(P)[None, :], dtype=np.float32
    ).astype(BF16NP)  # keep j<=i: rows p (j), cols u (i)

    nc = _get_nc()
    in_maps = []
    for c in range(8):
        b, k = divmod(c, 4)
        heads = [SLOT_BASE[s] + k for s in range(HPC)]
        aq = np.zeros((HPC, AUG, S), np.float32)
        for s, h in enumerate(heads):
            aq[s, 0, :] = slopes_bf[h]
            aq[s, 1, :] = slopes_bf[h]
            aq[s, 2, :] = -slopes_bf[h] * iv
        cols = np.concatenate(
            [np.arange(h * HWID, (h + 1) * HWID) for h in heads])
        in_maps.append({
            "xq": qTs[b], "xk": kTs[b], "xv": vTs[b],
            "wq": asc(WqT[:, cols]), "wk": asc(WkT[:, cols]),
            "wv": asc(WvT[:, cols]),
            "augq": aq.astype(BF16NP), "augk": augk_np,
            "tri": tri_np,
        })

    res = run_bass_kernel_spmd(nc, in_maps, core_ids=list(range(8)))
    outp = np.empty((B, S, D), np.float32)
    for c in range(8):
        b, k = divmod(c, 4)
        for s in range(HPC):
            h = SLOT_BASE[s] + k
            outp[b, :, h * HWID:(h + 1) * HWID] = \
                res.results[c]["out"][:, s * HWID:(s + 1) * HWID]
    return outp


# revision 39
# speedup vs baseline: 1.0571x; 1.0571x over previous
"""MHSA (dense transformer, ALiBi + causal) TRN2 Bass kernel, 8-core SPMD.

v4 design:
- Sharding: batch (2) x head-quartile -> 8 cores, heads REBALANCED so every
  core gets one head from each ALiBi-slope quartile: core c (b=c//4, k=c%4)
  computes heads [12+k, 8+k, 4+k, 0+k] (0-indexed) of batch b. Slot s on all
  cores then shares one causal band -> SPMD-friendly block skipping.
- Banded causal attention: ALiBi slope*(i-j) >~ 32 => weight < e^-32,
  skipped structurally. Bands (in 128-blocks) per slot: [16, 16, 5, 2].
- All matmuls bf16 (1 cyc/row on PE at any width). ALiBi folded into 3
  bf16-exact aug contraction rows:
    Q~ = [Q; slope; slope; -slope*i],  K~ = [K; j_hi; j_lo; 1]
  with j_hi multiple of 256 and j_lo in [0,256) - both exact in bf16; the
  -slope*i row is a per-row shift that cancels in softmax.
- Projection phase (x/w bf16, PSUM f32): Q^T/K^T copied to bf16 SBUF slot
  tiles by DVE, V by DVE into a [j, slot, 65] bf16 tile with a ones column
  (column 64 of the AV output becomes the softmax denominator).
- Attention interleaved with projections by i-window: window t = i in
  [512t, 512t+512), its strip groups round-robined with the projection
  chains of superblock t+1 so ACT exp overlaps PE projection matmuls.
- Scores S^T[j,i] per (slot, J): strips grouped into <=1024-col PSUM tiles,
  ONE exp (ACT) per group -> bf16 P^T; diagonal 128-strips masked on Pool
  (gpsimd) with a 0/1 triangle.
- AV: out[i(128), 65] accumulated over J into a [128, 4, 80] PSUM tile.
  IMPORTANT: PSUM accumulation groups are tracked per 2KB bank - only ONE
  chain may be open per bank at a time (interleaved starts lazily re-zero
  the bank). So all AV chains of a window are emitted at window END, one
  i-block at a time, each chain fully closed before the next starts.
- Epilogue = batched reciprocal + tensor_scalar multiplies (DVE) into an
  SBUF staging tile, DMA'd out per slot-pair (512B elements).
"""

import numpy as np
import ml_dtypes

import concourse.bass as bass
import concourse.mybir as mybir
import concourse.tile as tile
from concourse import bacc
from concourse.bass_utils import run_bass_kernel_spmd

P = 128
S = 2048
D = 1024
H = 16
HWID = 64
HPC = 4           # head slots per core
CW = HPC * HWID   # 256
NKC = D // P      # 8 contraction chunks
NTSB = 4          # t super-blocks (projection + attention i-windows)
TSB = S // NTSB   # 512
NJ = S // P       # 16
AUG = 3
QROWS = HWID + AUG  # 67
VSTR = 72           # V~ sub-block stride: 64 data + aligned ones lane
VONE = 64           # ones column (softmax denominator), first col of the lane
BANDS = [16, 16, 5, 2]      # causal band per slot, in 128-blocks
SLOT_BASE = [12, 8, 4, 0]   # head (0-indexed) = SLOT_BASE[s] + (core % 4)

F32 = mybir.dt.float32
BF16 = mybir.dt.bfloat16

EXP_BIAS = -12.0
BF16NP = ml_dtypes.bfloat16


def window_strips(s, t):
    """Strips (J, i0, i1) of window t for slot s (banded causal)."""
    B = BANDS[s]
    res = []
    for J in range(max(0, 4 * t - B + 1), 4 * t + 4):
        i0 = max(TSB * t, P * J)
        i1 = min(TSB * t + TSB, P * (J + B), S)
        if i1 > i0:
            res.append((J, i0, i1))
    return res


def group_strips(strips_w, cap=1024):
    groups, cur, w = [], [], 0
    for (J, i0, i1) in strips_w:
        if w + (i1 - i0) > cap and cur:
            groups.append(cur)
            cur, w = [], 0
        cur.append((J, i0, i1))
        w += i1 - i0
    if cur:
        groups.append(cur)
    return groups


def build_kernel():
    nc = bacc.Bacc("TRN2")

    xq = nc.dram_tensor("xq", [D, S], BF16, kind="ExternalInput")
    xk = nc.dram_tensor("xk", [D, S], BF16, kind="ExternalInput")
    xv = nc.dram_tensor("xv", [D, S], BF16, kind="ExternalInput")
    wq = nc.dram_tensor("wq", [D, CW], BF16, kind="ExternalInput")
    wk = nc.dram_tensor("wk", [D, CW], BF16, kind="ExternalInput")
    wv = nc.dram_tensor("wv", [D, CW], BF16, kind="ExternalInput")
    augq = nc.dram_tensor("augq", [HPC, AUG, S], BF16, kind="ExternalInput")
    augk = nc.dram_tensor("augk", [AUG, S], BF16, kind="ExternalInput")
    tri = nc.dram_tensor("tri", [P, P], BF16, kind="ExternalInput")
    out = nc.dram_tensor("out", [S, CW], F32, kind="ExternalOutput")

    x_drams = [xq, xk, xv]
    w_drams = [wq, wk, wv]

    with tile.TileContext(nc) as tc:
        with (
            tc.tile_pool(name="cn", bufs=1) as cnp,
            tc.tile_pool(name="wp", bufs=1) as wp,
            tc.tile_pool(name="qk", bufs=1) as qkp,
            tc.tile_pool(name="vv", bufs=1) as vvp,
            tc.tile_pool(name="xp", bufs=3) as xp,
            tc.tile_pool(name="pt", bufs=12) as ptp,
            tc.tile_pool(name="rc", bufs=3) as rcp,
            tc.tile_pool(name="ob", bufs=1) as obp,
            tc.tile_pool(name="pq", bufs=2, space="PSUM") as pqp,
            tc.tile_pool(name="sc", bufs=2, space="PSUM") as scp,
            tc.tile_pool(name="av", bufs=2, space="PSUM") as avp,
        ):
            # ---- startup DMAs in critical-path order: (wv, xv0), (wq, xq0),
            # slot-0/1 augs, (wk, xk0), slot-2/3 augs — V/Q chains and the
            # first windows (slots 0/1) start as soon as their data lands ----
            def emit_xdma_one(xt, t, xi):
                nc.sync.dma_start(
                    xt[:, :, xi, :],
                    x_drams[xi].rearrange("(ko p) t -> p ko t", p=P)[
                        :, :, t * TSB:(t + 1) * TSB],
                )

            def emit_xdma(t):
                xt = xp.tile([P, NKC, 3, TSB], BF16, tag="x", name=f"x{t}")
                for xi in range(3):
                    emit_xdma_one(xt, t, xi)
                return xt

            w_tiles = [None, None, None]
            xt0 = xp.tile([P, NKC, 3, TSB], BF16, tag="x", name="x0")

            def emit_wdma(i):
                wt = wp.tile([P, NKC, CW], BF16, tag=f"w{i}", name=f"w{i}")
                nc.sync.dma_start(
                    wt[:], w_drams[i].rearrange("(ko p) c -> p ko c", p=P))
                w_tiles[i] = wt

            emit_wdma(2)
            emit_xdma_one(xt0, 0, 2)
            emit_wdma(0)
            emit_xdma_one(xt0, 0, 0)

            # ---- constants ----
            tri_t = cnp.tile([P, P], BF16, tag="tri", name="tri_t")
            nc.sync.dma_start(tri_t[:], tri[:])
            ebias = cnp.tile([P, 1], F32, tag="ebias", name="ebias")
            nc.gpsimd.memset(ebias[:], EXP_BIAS)

            # ---- per-slot Q~^T / K~^T [67, S] bf16; V~ [128, NJ, HPC, 65] ----
            qs = [qkp.tile([QROWS, S], BF16, tag=f"q{s}", name=f"q{s}")
                  for s in range(HPC)]
            ks = [qkp.tile([QROWS, S], BF16, tag=f"k{s}", name=f"k{s}")
                  for s in range(HPC)]
            # V~ sub-block = 72 cols: V data 0:64, then a 16-byte aligned
            # constant lane [64:72) holding the softmax-denominator ones at
            # col VONE. Padding keeps any >=4-byte RMW write granule away
            # from the bf16 V data (same-engine memsets, ordered before the
            # DVE V copies).
            v_t = vvp.tile([P, NJ, HPC, VSTR], BF16, tag="v", name="v_t")
            nc.vector.memset(v_t[:, :, :, HWID:VSTR], 0.0)
            nc.vector.memset(v_t[:, :, :, VONE:VONE + 1], 1.0)

            # slot-0/1 aug rows early (first windows), then (wk, xk0),
            # then slot-2/3 augs
            for s in range(2):
                nc.sync.dma_start(qs[s][HWID:QROWS, :], augq[s])
                nc.sync.dma_start(ks[s][HWID:QROWS, :], augk[:])
            emit_wdma(1)
            emit_xdma_one(xt0, 0, 1)
            for s in range(2, HPC):
                nc.sync.dma_start(qs[s][HWID:QROWS, :], augq[s])
                nc.sync.dma_start(ks[s][HWID:QROWS, :], augk[:])

            # ---- output staging [P, NJ, CW] f32 ----
            out_sb = obp.tile([P, NJ, CW], F32, tag="ob", name="out_sb")

            # ---------- emission helpers ----------
            def proj_chain_units(t, xt):
                units = []
                if t == 0:
                    # V first (xv lands first), then Q/K cc0 (slots 0/1 ->
                    # first windows), then cc1
                    for u in range(TSB // P):
                        units.append(("v", t, xt, u))
                    for cc in range(2):
                        for pi in range(2):
                            units.append(("qk", t, xt, pi, cc))
                    return units
                for pi in range(2):
                    for cc in range(2):
                        units.append(("qk", t, xt, pi, cc))
                for u in range(TSB // P):
                    units.append(("v", t, xt, u))
                return units

            def emit_chain(unit):
                kind = unit[0]
                if kind == "qk":
                    _, t, xt, pi, cc = unit
                    dsts = qs if pi == 0 else ks
                    ps = pqp.tile([P, TSB], F32, tag="pq",
                                  name=f"pq{t}_{pi}_{cc}")
                    for kk in range(NKC):
                        nc.tensor.matmul(
                            ps[:],
                            lhsT=w_tiles[pi][:, kk, cc * P:(cc + 1) * P],
                            rhs=xt[:, kk, pi, :],
                            start=(kk == 0),
                            stop=(kk == NKC - 1),
                        )
                    nc.vector.tensor_copy(
                        dsts[2 * cc][0:HWID, t * TSB:(t + 1) * TSB],
                        ps[0:HWID, :],
                    )
                    nc.vector.tensor_copy(
                        dsts[2 * cc + 1][0:HWID, t * TSB:(t + 1) * TSB],
                        ps[HWID:P, :],
                    )
                else:
                    _, t, xt, u = unit
                    tt = t * (TSB // P) + u
                    ps = pqp.tile([P, TSB], F32, tag="pq", name=f"pv{t}_{u}")
                    for kk in range(NKC):
                        nc.tensor.matmul(
                            ps[:, 0:CW],
                            lhsT=xt[:, kk, 2, u * P:(u + 1) * P],
                            rhs=w_tiles[2][:, kk, :],
                            start=(kk == 0),
                            stop=(kk == NKC - 1),
                        )
                    nc.vector.tensor_copy(
                        v_t[:, tt, :, 0:HWID],
                        ps[:, 0:CW].rearrange("p (h w) -> p h w", h=HPC),
                    )

            def emit_scores_exp(s, t, g, gi):
                width = sum(i1 - i0 for (_, i0, i1) in g)
                sc = scp.tile([P, 1024], F32, tag="sc", name=f"sc{s}_{t}_{gi}")
                o = 0
                offs = []
                for (J, i0, i1) in g:
                    W = i1 - i0
                    a = 0
                    while a < W:  # split at psum bank boundaries (512 cols)
                        b = min(W, a + 512 - (o + a) % 512)
                        nc.tensor.matmul(
                            sc[:, o + a:o + b],
                            lhsT=ks[s][0:QROWS, P * J:P * (J + 1)],
                            rhs=qs[s][0:QROWS, i0 + a:i0 + b],
                            start=True,
                            stop=True,
                        )
                        a = b
                    offs.append((J, i0, i1, o))
                    o += W
                pt = ptp.tile([P, 1024], BF16, tag="pt", name=f"pt{s}_{t}_{gi}")
                nc.scalar.activation(
                    pt[:, 0:width], sc[:, 0:width],
                    mybir.ActivationFunctionType.Exp,
                    bias=ebias[:], scale=1.0,
                )
                for (J, i0, i1, off) in offs:
                    if i0 == P * J:
                        # mask the diagonal 128-strip (keep j <= i), on Pool
                        nc.gpsimd.tensor_mul(
                            pt[:, off:off + P], pt[:, off:off + P], tri_t[:]
                        )
                return pt, offs

            def emit_epilogue(s, t, acc):
                rec = rcp.tile([P, 4, 1], F32, tag="rc", name=f"rc{s}_{t}")
                nc.vector.reciprocal(rec[:], acc[:, :, VONE:VONE + 1])
                for r in range(4):
                    nc.vector.tensor_scalar_mul(
                        out_sb[:, 4 * t + r, s * HWID:(s + 1) * HWID],
                        acc[:, r, 0:HWID],
                        rec[:, r, :],
                    )
                if s % 2 == 1 and t in (NTSB // 2 - 1, NTSB - 1):
                    # slot pair: 128 contiguous f32 columns -> 512B elems;
                    # rows [0,1024) leave early (after t=1), only the upper
                    # half remains in the tail
                    a0 = 0 if t == NTSB // 2 - 1 else NJ // 2
                    a1 = NJ // 2 if t == NTSB // 2 - 1 else NJ
                    nc.sync.dma_start(
                        out.rearrange("(a p) c -> p a c", p=P)[
                            :, a0:a1, (s - 1) * HWID:(s + 1) * HWID],
                        out_sb[:, a0:a1, (s - 1) * HWID:(s + 1) * HWID],
                    )

            # pending completed window: (s, t, acc, [(pt, offs), ...])
            pend_w = None
            pend_age = 0  # groups emitted since pend_w was set

            def flush_window():
                nonlocal pend_w
                if pend_w is None:
                    return
                s, t, acc, recs = pend_w
                B = BANDS[s]
                jmap = {}
                for pt, offs in recs:
                    for (J, i0, i1, off) in offs:
                        jmap[J] = (pt, off, i0)
                # one fully-closed accumulation chain per i-block (PSUM bank
                # allows only one open chain at a time)
                for r in range(4):
                    i_blk = 4 * t + r
                    jst = max(0, i_blk - B + 1)
                    for J in range(jst, i_blk + 1):
                        pt, off, i0 = jmap[J]
                        col = off + i_blk * P - i0
                        nc.tensor.matmul(
                            acc[:, r, 0:VONE + 1],
                            lhsT=pt[:, col:col + P],
                            rhs=v_t[:, J, s, 0:VONE + 1],
                            start=(J == jst),
                            stop=(J == i_blk),
                        )
                emit_epilogue(s, t, acc)
                pend_w = None

            # ---------- main schedule ----------
            for unit in proj_chain_units(0, xt0):
                emit_chain(unit)

            for t in range(NTSB):
                if t + 1 < NTSB:
                    xt_next = emit_xdma(t + 1)
                    next_chains = proj_chain_units(t + 1, xt_next)
                else:
                    next_chains = []
                gitems = []
                for s in range(HPC):
                    gitems.append((s, group_strips(window_strips(s, t))))
                total_groups = sum(len(g) for _, g in gitems)
                ci = 0
                gcount = 0
                for s, groups in gitems:
                    acc = avp.tile([P, 4, 80], F32, tag="acc",
                                   name=f"acc{s}_{t}")
                    recs = []
                    for gi, g in enumerate(groups):
                        pt, offs = emit_scores_exp(s, t, g, gi)
                        recs.append((pt, offs))
                        gcount += 1
                        pend_age += 1
                        while (ci < len(next_chains)
                               and ci < (gcount * len(next_chains))
                               // total_groups):
                            emit_chain(next_chains[ci])
                            ci += 1
                        if pend_age >= 2:
                            flush_window()
                    flush_window()  # ensure previous window drained
                    pend_w = (s, t, acc, recs)
                    pend_age = 0
                while ci < len(next_chains):
                    emit_chain(next_chains[ci])
                    ci += 1
            flush_window()

    nc.compile()
    return nc


_NC = None


def _get_nc():
    global _NC
    if _NC is None:
        _NC = build_kernel()
    return _NC


def kernel(queries, keys, values, mask, Wq, Wk, Wv):
    B = queries.shape[0]
    asc = np.ascontiguousarray
    scale = 1.0 / np.sqrt(HWID)

    WqT = asc((np.asarray(Wq).T * scale).astype(np.float32)).astype(BF16NP)
    WkT = asc(np.asarray(Wk).T.astype(np.float32)).astype(BF16NP)
    WvT = asc(np.asarray(Wv).T.astype(np.float32)).astype(BF16NP)
    qTs = [asc(np.asarray(queries[b]).T.astype(np.float32)).astype(BF16NP)
           for b in range(B)]
    kTs = [asc(np.asarray(keys[b]).T.astype(np.float32)).astype(BF16NP)
           for b in range(B)]
    vTs = [asc(np.asarray(values[b]).T.astype(np.float32)).astype(BF16NP)
           for b in range(B)]

    slopes = (2.0 ** (-np.arange(1, H + 1) * (8.0 / H))).astype(np.float32)
    slopes_bf = slopes.astype(BF16NP).astype(np.float32)
    iv = np.arange(S, dtype=np.float32)
    j_hi = (np.arange(S) // 256 * 256).astype(np.float32)
    j_lo = (np.arange(S) % 256).astype(np.float32)
    augk_np = np.stack([j_hi, j_lo, np.ones(S, np.float32)]).astype(BF16NP)
    tri_np = np.asarray(
        np.arange(P)[:, None] <= np.arange(P)[None, :], dtype=np.float32
    ).astype(BF16NP)  # keep j<=i: rows p (j), cols u (i)

    nc = _get_nc()
    in_maps = []
    for c in range(8):
        b, k = divmod(c, 4)
        heads = [SLOT_BASE[s] + k for s in range(HPC)]
        aq = np.zeros((HPC, AUG, S), np.float32)
        for s, h in enumerate(heads):
            aq[s, 0, :] = slopes_bf[h]
            aq[s, 1, :] = slopes_bf[h]
            aq[s, 2, :] = -slopes_bf[h] * iv
        cols = np.concatenate(
            [np.arange(h * HWID, (h + 1) * HWID) for h in heads])
        in_maps.append({
            "xq": qTs[b], "xk": kTs[b], "xv": vTs[b],
            "wq": asc(WqT[:, cols]), "wk": asc(WkT[:, cols]),
            "wv": asc(WvT[:, cols]),
            "augq": aq.astype(BF16NP), "augk": augk_np,
            "tri": tri_np,
        })

    res = run_bass_kernel_spmd(nc, in_maps, core_ids=list(range(8)))
    outp = np.empty((B, S, D), np.float32)
    for c in range(8):
        b, k = divmod(c, 4)
        for s in range(HPC):
            h = SLOT_BASE[s] + k
            outp[b, :, h * HWID:(h + 1) * HWID] = \
                res.results[c]["out"][:, s * HWID:(s + 1) * HWID]
    return outp


# revision 41
# speedup vs baseline: 1.0665x; 1.0090x over previous
"""MHSA (dense transformer, ALiBi + causal) TRN2 Bass kernel, 8-core SPMD.

v4 design:
- Sharding: batch (2) x head-quartile -> 8 cores, heads REBALANCED so every
  core gets one head from each ALiBi-slope quartile: core c (b=c//4, k=c%4)
  computes heads [12+k, 8+k, 4+k, 0+k] (0-indexed) of batch b. Slot s on all
  cores then shares one causal band -> SPMD-friendly block skipping.
- Banded causal attention: ALiBi slope*(i-j) >~ 32 => weight < e^-32,
  skipped structurally. Bands (in 128-blocks) per slot: [16, 16, 5, 2].
- All matmuls bf16 (1 cyc/row on PE at any width). ALiBi folded into 3
  bf16-exact aug contraction rows:
    Q~ = [Q; slope; slope; -slope*i],  K~ = [K; j_hi; j_lo; 1]
  with j_hi multiple of 256 and j_lo in [0,256) - both exact in bf16; the
  -slope*i row is a per-row shift that cancels in softmax.
- Projection phase (x/w bf16, PSUM f32): Q^T/K^T copied to bf16 SBUF slot
  tiles by DVE, V by DVE into a [j, slot, 65] bf16 tile with a ones column
  (column 64 of the AV output becomes the softmax denominator).
- Attention interleaved with projections by i-window: window t = i in
  [512t, 512t+512), its strip groups round-robined with the projection
  chains of superblock t+1 so ACT exp overlaps PE projection matmuls.
- Scores S^T[j,i] per (slot, J): strips grouped into <=1024-col PSUM tiles,
  ONE exp (ACT) per group -> bf16 P^T; diagonal 128-strips masked on Pool
  (gpsimd) with a 0/1 triangle.
- AV: out[i(128), 65] accumulated over J into a [128, 4, 80] PSUM tile.
  IMPORTANT: PSUM accumulation groups are tracked per 2KB bank - only ONE
  chain may be open per bank at a time (interleaved starts lazily re-zero
  the bank). So all AV chains of a window are emitted at window END, one
  i-block at a time, each chain fully closed before the next starts.
- Epilogue = batched reciprocal + tensor_scalar multiplies (DVE) into an
  SBUF staging tile, DMA'd out per slot-pair (512B elements).
"""

import numpy as np
import ml_dtypes

import concourse.bass as bass
import concourse.mybir as mybir
import concourse.tile as tile
from concourse import bacc
from concourse.bass_utils import run_bass_kernel_spmd

P = 128
S = 2048
D = 1024
H = 16
HWID = 64
HPC = 4           # head slots per core
CW = HPC * HWID   # 256
NKC = D // P      # 8 contraction chunks
NTSB = 4          # t super-blocks (projection + attention i-windows)
TSB = S // NTSB   # 512
NJ = S // P       # 16
AUG = 3
QROWS = HWID + AUG  # 67
VSTR = 72           # V~ sub-block stride: 64 data + aligned ones lane
VONE = 64           # ones column (softmax denominator), first col of the lane
BANDS = [16, 16, 5, 2]      # causal band per slot, in 128-blocks
SLOT_BASE = [12, 8, 4, 0]   # head (0-indexed) = SLOT_BASE[s] + (core % 4)

F32 = mybir.dt.float32
BF16 = mybir.dt.bfloat16

EXP_BIAS = -12.0
BF16NP = ml_dtypes.bfloat16


def window_strips(s, t):
    """Strips (J, i0, i1) of window t for slot s (banded causal)."""
    B = BANDS[s]
    res = []
    for J in range(max(0, 4 * t - B + 1), 4 * t + 4):
        i0 = max(TSB * t, P * J)
        i1 = min(TSB * t + TSB, P * (J + B), S)
        if i1 > i0:
            res.append((J, i0, i1))
    return res


def group_strips(strips_w, cap=1024):
    groups, cur, w = [], [], 0
    for (J, i0, i1) in strips_w:
        if w + (i1 - i0) > cap and cur:
            groups.append(cur)
            cur, w = [], 0
        cur.append((J, i0, i1))
        w += i1 - i0
    if cur:
        groups.append(cur)
    return groups


def build_kernel():
    nc = bacc.Bacc("TRN2")

    xq = nc.dram_tensor("xq", [D, S], BF16, kind="ExternalInput")
    xk = nc.dram_tensor("xk", [D, S], BF16, kind="ExternalInput")
    xv = nc.dram_tensor("xv", [D, S], BF16, kind="ExternalInput")
    wq = nc.dram_tensor("wq", [D, CW], BF16, kind="ExternalInput")
    wk = nc.dram_tensor("wk", [D, CW], BF16, kind="ExternalInput")
    wv = nc.dram_tensor("wv", [D, CW], BF16, kind="ExternalInput")
    augq = nc.dram_tensor("augq", [HPC, AUG, S], BF16, kind="ExternalInput")
    augk = nc.dram_tensor("augk", [AUG, S], BF16, kind="ExternalInput")
    tri = nc.dram_tensor("tri", [P, P], BF16, kind="ExternalInput")
    out = nc.dram_tensor("out", [S, CW], F32, kind="ExternalOutput")

    x_drams = [xq, xk, xv]
    w_drams = [wq, wk, wv]

    with tile.TileContext(nc) as tc:
        with (
            tc.tile_pool(name="cn", bufs=1) as cnp,
            tc.tile_pool(name="wp", bufs=1) as wp,
            tc.tile_pool(name="qk", bufs=1) as qkp,
            tc.tile_pool(name="vv", bufs=1) as vvp,
            tc.tile_pool(name="xp", bufs=3) as xp,
            tc.tile_pool(name="pt", bufs=16) as ptp,
            tc.tile_pool(name="rc", bufs=3) as rcp,
            tc.tile_pool(name="ob", bufs=1) as obp,
            tc.tile_pool(name="pq", bufs=2, space="PSUM") as pqp,
            tc.tile_pool(name="sc", bufs=2, space="PSUM") as scp,
            tc.tile_pool(name="av", bufs=2, space="PSUM") as avp,
        ):
            # ---- startup DMAs in critical-path order: (wv, xv0), (wq, xq0),
            # slot-0/1 augs, (wk, xk0), slot-2/3 augs — V/Q chains and the
            # first windows (slots 0/1) start as soon as their data lands ----
            def emit_xdma_one(xt, t, xi):
                nc.sync.dma_start(
                    xt[:, :, xi, :],
                    x_drams[xi].rearrange("(ko p) t -> p ko t", p=P)[
                        :, :, t * TSB:(t + 1) * TSB],
                )

            def emit_xdma(t):
                xt = xp.tile([P, NKC, 3, TSB], BF16, tag="x", name=f"x{t}")
                for xi in range(3):
                    emit_xdma_one(xt, t, xi)
                return xt

            w_tiles = [None, None, None]
            xt0 = xp.tile([P, NKC, 3, TSB], BF16, tag="x", name="x0")

            def emit_wdma(i):
                wt = wp.tile([P, NKC, CW], BF16, tag=f"w{i}", name=f"w{i}")
                nc.sync.dma_start(
                    wt[:], w_drams[i].rearrange("(ko p) c -> p ko c", p=P))
                w_tiles[i] = wt

            emit_wdma(2)
            emit_xdma_one(xt0, 0, 2)
            emit_wdma(0)
            emit_xdma_one(xt0, 0, 0)

            # ---- constants ----
            tri_t = cnp.tile([P, P], BF16, tag="tri", name="tri_t")
            nc.sync.dma_start(tri_t[:], tri[:])
            ebias = cnp.tile([P, 1], F32, tag="ebias", name="ebias")
            nc.gpsimd.memset(ebias[:], EXP_BIAS)

            # ---- per-slot Q~^T / K~^T [67, S] bf16; V~ [128, NJ, HPC, 65] ----
            qs = [qkp.tile([QROWS, S], BF16, tag=f"q{s}", name=f"q{s}")
                  for s in range(HPC)]
            ks = [qkp.tile([QROWS, S], BF16, tag=f"k{s}", name=f"k{s}")
                  for s in range(HPC)]
            # V~ sub-block = 72 cols: V data 0:64, then a 16-byte aligned
            # constant lane [64:72) holding the softmax-denominator ones at
            # col VONE. Padding keeps any >=4-byte RMW write granule away
            # from the bf16 V data (same-engine memsets, ordered before the
            # DVE V copies).
            v_t = vvp.tile([P, NJ, HPC, VSTR], BF16, tag="v", name="v_t")
            nc.vector.memset(v_t[:, :, :, HWID:VSTR], 0.0)
            nc.vector.memset(v_t[:, :, :, VONE:VONE + 1], 1.0)

            # slot-0/1 aug rows early (first windows), then (wk, xk0),
            # then slot-2/3 augs
            for s in range(2):
                nc.sync.dma_start(qs[s][HWID:QROWS, :], augq[s])
                nc.sync.dma_start(ks[s][HWID:QROWS, :], augk[:])
            emit_wdma(1)
            emit_xdma_one(xt0, 0, 1)
            for s in range(2, HPC):
                nc.sync.dma_start(qs[s][HWID:QROWS, :], augq[s])
                nc.sync.dma_start(ks[s][HWID:QROWS, :], augk[:])

            # ---- output staging [P, NJ, CW] f32 ----
            out_sb = obp.tile([P, NJ, CW], F32, tag="ob", name="out_sb")

            # ---------- emission helpers ----------
            def proj_chain_units(t, xt):
                units = []
                if t == 0:
                    # V first (xv lands first), then Q/K cc0 (slots 0/1 ->
                    # first windows), then cc1
                    for u in range(TSB // P):
                        units.append(("v", t, xt, u))
                    for cc in range(2):
                        for pi in range(2):
                            units.append(("qk", t, xt, pi, cc))
                    return units
                for pi in range(2):
                    for cc in range(2):
                        units.append(("qk", t, xt, pi, cc))
                for u in range(TSB // P):
                    units.append(("v", t, xt, u))
                return units

            def emit_chain(unit):
                kind = unit[0]
                if kind == "qk":
                    _, t, xt, pi, cc = unit
                    dsts = qs if pi == 0 else ks
                    ps = pqp.tile([P, TSB], F32, tag="pq",
                                  name=f"pq{t}_{pi}_{cc}")
                    for kk in range(NKC):
                        nc.tensor.matmul(
                            ps[:],
                            lhsT=w_tiles[pi][:, kk, cc * P:(cc + 1) * P],
                            rhs=xt[:, kk, pi, :],
                            start=(kk == 0),
                            stop=(kk == NKC - 1),
                        )
                    nc.vector.tensor_copy(
                        dsts[2 * cc][0:HWID, t * TSB:(t + 1) * TSB],
                        ps[0:HWID, :],
                    )
                    nc.vector.tensor_copy(
                        dsts[2 * cc + 1][0:HWID, t * TSB:(t + 1) * TSB],
                        ps[HWID:P, :],
                    )
                else:
                    _, t, xt, u = unit
                    tt = t * (TSB // P) + u
                    ps = pqp.tile([P, TSB], F32, tag="pq", name=f"pv{t}_{u}")
                    for kk in range(NKC):
                        nc.tensor.matmul(
                            ps[:, 0:CW],
                            lhsT=xt[:, kk, 2, u * P:(u + 1) * P],
                            rhs=w_tiles[2][:, kk, :],
                            start=(kk == 0),
                            stop=(kk == NKC - 1),
                        )
                    nc.vector.tensor_copy(
                        v_t[:, tt, :, 0:HWID],
                        ps[:, 0:CW].rearrange("p (h w) -> p h w", h=HPC),
                    )

            def emit_scores_exp(s, t, g, gi):
                width = sum(i1 - i0 for (_, i0, i1) in g)
                sc = scp.tile([P, 1024], F32, tag="sc", name=f"sc{s}_{t}_{gi}")
                o = 0
                offs = []
                for (J, i0, i1) in g:
                    W = i1 - i0
                    a = 0
                    while a < W:  # split at psum bank boundaries (512 cols)
                        b = min(W, a + 512 - (o + a) % 512)
                        nc.tensor.matmul(
                            sc[:, o + a:o + b],
                            lhsT=ks[s][0:QROWS, P * J:P * (J + 1)],
                            rhs=qs[s][0:QROWS, i0 + a:i0 + b],
                            start=True,
                            stop=True,
                        )
                        a = b
                    offs.append((J, i0, i1, o))
                    o += W
                pt = ptp.tile([P, 1024], BF16, tag="pt", name=f"pt{s}_{t}_{gi}")
                nc.scalar.activation(
                    pt[:, 0:width], sc[:, 0:width],
                    mybir.ActivationFunctionType.Exp,
                    bias=ebias[:], scale=1.0,
                )
                for (J, i0, i1, off) in offs:
                    if i0 == P * J:
                        # mask the diagonal 128-strip (keep j <= i), on Pool
                        nc.gpsimd.tensor_mul(
                            pt[:, off:off + P], pt[:, off:off + P], tri_t[:]
                        )
                return pt, offs

            def emit_epilogue(s, t, acc):
                rec = rcp.tile([P, 4, 1], F32, tag="rc", name=f"rc{s}_{t}")
                nc.vector.reciprocal(rec[:], acc[:, :, VONE:VONE + 1])
                for r in range(4):
                    nc.vector.tensor_scalar_mul(
                        out_sb[:, 4 * t + r, s * HWID:(s + 1) * HWID],
                        acc[:, r, 0:HWID],
                        rec[:, r, :],
                    )
                if s % 2 == 1 and t in (NTSB // 2 - 1, NTSB - 1):
                    # slot pair: 128 contiguous f32 columns -> 512B elems;
                    # rows [0,1024) leave early (after t=1), only the upper
                    # half remains in the tail
                    a0 = 0 if t == NTSB // 2 - 1 else NJ // 2
                    a1 = NJ // 2 if t == NTSB // 2 - 1 else NJ
                    nc.sync.dma_start(
                        out.rearrange("(a p) c -> p a c", p=P)[
                            :, a0:a1, (s - 1) * HWID:(s + 1) * HWID],
                        out_sb[:, a0:a1, (s - 1) * HWID:(s + 1) * HWID],
                    )

            # pending completed window: (s, t, acc, [(pt, offs), ...])
            pend_w = None
            pend_age = 0  # groups emitted since pend_w was set

            def flush_window():
                nonlocal pend_w
                if pend_w is None:
                    return
                s, t, acc, recs = pend_w
                B = BANDS[s]
                jmap = {}
                for pt, offs in recs:
                    for (J, i0, i1, off) in offs:
                        jmap[J] = (pt, off, i0)
                # one fully-closed accumulation chain per i-block (PSUM bank
                # allows only one open chain at a time)
                for r in range(4):
                    i_blk = 4 * t + r
                    jst = max(0, i_blk - B + 1)
                    for J in range(jst, i_blk + 1):
                        pt, off, i0 = jmap[J]
                        col = off + i_blk * P - i0
                        nc.tensor.matmul(
                            acc[:, r, 0:VONE + 1],
                            lhsT=pt[:, col:col + P],
                            rhs=v_t[:, J, s, 0:VONE + 1],
                            start=(J == jst),
                            stop=(J == i_blk),
                        )
                emit_epilogue(s, t, acc)
                pend_w = None

            # ---------- main schedule ----------
            for unit in proj_chain_units(0, xt0):
                emit_chain(unit)

            for t in range(NTSB):
                if t + 1 < NTSB:
                    xt_next = emit_xdma(t + 1)
                    next_chains = proj_chain_units(t + 1, xt_next)
                else:
                    next_chains = []
                gitems = []
                for s in range(HPC):
                    gitems.append((s, group_strips(window_strips(s, t))))
                total_groups = sum(len(g) for _, g in gitems)
                ci = 0
                gcount = 0
                for s, groups in gitems:
                    acc = avp.tile([P, 4, 80], F32, tag="acc",
                                   name=f"acc{s}_{t}")
                    recs = []
                    for gi, g in enumerate(groups):
                        pt, offs = emit_scores_exp(s, t, g, gi)
                        recs.append((pt, offs))
                        gcount += 1
                        pend_age += 1
                        while (ci < len(next_chains)
                               and ci < (gcount * len(next_chains))
                               // total_groups):
                            emit_chain(next_chains[ci])
                            ci += 1
                        if pend_age >= 3:
                            flush_window()
                    flush_window()  # ensure previous window drained
                    pend_w = (s, t, acc, recs)
                    pend_age = 0
                while ci < len(next_chains):
                    emit_chain(next_chains[ci])
                    ci += 1
            flush_window()

    nc.compile()
    return nc


_NC = None


def _get_nc():
    global _NC
    if _NC is None:
        _NC = build_kernel()
    return _NC


def kernel(queries, keys, values, mask, Wq, Wk, Wv):
    B = queries.shape[0]
    asc = np.ascontiguousarray
    scale = 1.0 / np.sqrt(HWID)

    WqT = asc((np.asarray(Wq).T * scale).astype(np.float32)).astype(BF16NP)
    WkT = asc(np.asarray(Wk).T.astype(np.float32)).astype(BF16NP)
    WvT = asc(np.asarray(Wv).T.astype(np.float32)).astype(BF16NP)
    qTs = [asc(np.asarray(queries[b]).T.astype(np.float32)).astype(BF16NP)
           for b in range(B)]
    kTs = [asc(np.asarray(keys[b]).T.astype(np.float32)).astype(BF16NP)
           for b in range(B)]
    vTs = [asc(np.asarray(values[b]).T.astype(np.float32)).astype(BF16NP)
           for b in range(B)]

    slopes = (2.0 ** (-np.arange(1, H + 1) * (8.0 / H))).astype(np.float32)
    slopes_bf = slopes.astype(BF16NP).astype(np.float32)
    iv = np.arange(S, dtype=np.float32)
    j_hi = (np.arange(S) // 256 * 256).astype(np.float32)
    j_lo = (np.arange(S) % 256).astype(np.float32)
    augk_np = np.stack([j_hi, j_lo, np.ones(S, np.float32)]).astype(BF16NP)
    tri_np = np.asarray(
        np.arange(P)[:, None] <= np.arange(P)[None, :], dtype=np.float32
    ).astype(BF16NP)  # keep j<=i: rows p (j), cols u (i)

    nc = _get_nc()
    in_maps = []
    for c in range(8):
        b, k = divmod(c, 4)
        heads = [SLOT_BASE[s] + k for s in range(HPC)]
        aq = np.zeros((HPC, AUG, S), np.float32)
        for s, h in enumerate(heads):
            aq[s, 0, :] = slopes_bf[h]
            aq[s, 1, :] = slopes_bf[h]
            aq[s, 2, :] = -slopes_bf[h] * iv
        cols = np.concatenate(
            [np.arange(h * HWID, (h + 1) * HWID) for h in heads])
        in_maps.append({
            "xq": qTs[b], "xk": kTs[b], "xv": vTs[b],
            "wq": asc(WqT[:, cols]), "wk": asc(WkT[:, cols]),
            "wv": asc(WvT[:, cols]),
            "augq": aq.astype(BF16NP), "augk": augk_np,
            "tri": tri_np,
        })

    res = run_bass_kernel_spmd(nc, in_maps, core_ids=list(range(8)))
    outp = np.empty((B, S, D), np.float32)
    for c in range(8):
        b, k = divmod(c, 4)
        for s in range(HPC):
            h = SLOT_BASE[s] + k
            outp[b, :, h * HWID:(h + 1) * HWID] = \
                res.results[c]["out"][:, s * HWID:(s + 1) * HWID]
    return outp


# revision 42
# speedup vs baseline: 1.0733x; 1.0064x over previous
"""MHSA (dense transformer, ALiBi + causal) TRN2 Bass kernel, 8-core SPMD.

v4 design:
- Sharding: batch (2) x head-quartile -> 8 cores, heads REBALANCED so every
  core gets one head from each ALiBi-slope quartile: core c (b=c//4, k=c%4)
  computes heads [12+k, 8+k, 4+k, 0+k] (0-indexed) of batch b. Slot s on all
  cores then shares one causal band -> SPMD-friendly block skipping.
- Banded causal attention: ALiBi slope*(i-j) >~ 32 => weight < e^-32,
  skipped structurally. Bands (in 128-blocks) per slot: [16, 16, 5, 2].
- All matmuls bf16 (1 cyc/row on PE at any width). ALiBi folded into 3
  bf16-exact aug contraction rows:
    Q~ = [Q; slope; slope; -slope*i],  K~ = [K; j_hi; j_lo; 1]
  with j_hi multiple of 256 and j_lo in [0,256) - both exact in bf16; the
  -slope*i row is a per-row shift that cancels in softmax.
- Projection phase (x/w bf16, PSUM f32): Q^T/K^T copied to bf16 SBUF slot
  tiles by DVE, V by DVE into a [j, slot, 65] bf16 tile with a ones column
  (column 64 of the AV output becomes the softmax denominator).
- Attention interleaved with projections by i-window: window t = i in
  [512t, 512t+512), its strip groups round-robined with the projection
  chains of superblock t+1 so ACT exp overlaps PE projection matmuls.
- Scores S^T[j,i] per (slot, J): strips grouped into <=1024-col PSUM tiles,
  ONE exp (ACT) per group -> bf16 P^T; diagonal 128-strips masked on Pool
  (gpsimd) with a 0/1 triangle.
- AV: out[i(128), 65] accumulated over J into a [128, 4, 80] PSUM tile.
  IMPORTANT: PSUM accumulation groups are tracked per 2KB bank - only ONE
  chain may be open per bank at a time (interleaved starts lazily re-zero
  the bank). So all AV chains of a window are emitted at window END, one
  i-block at a time, each chain fully closed before the next starts.
- Epilogue = batched reciprocal + tensor_scalar multiplies (DVE) into an
  SBUF staging tile, DMA'd out per slot-pair (512B elements).
"""

import numpy as np
import ml_dtypes

import concourse.bass as bass
import concourse.mybir as mybir
import concourse.tile as tile
from concourse import bacc
from concourse.bass_utils import run_bass_kernel_spmd

P = 128
S = 2048
D = 1024
H = 16
HWID = 64
HPC = 4           # head slots per core
CW = HPC * HWID   # 256
NKC = D // P      # 8 contraction chunks
NTSB = 4          # t super-blocks (projection + attention i-windows)
TSB = S // NTSB   # 512
NJ = S // P       # 16
AUG = 3
QROWS = HWID + AUG  # 67
VSTR = 72           # V~ sub-block stride: 64 data + aligned ones lane
VONE = 64           # ones column (softmax denominator), first col of the lane
BANDS = [16, 16, 5, 2]      # causal band per slot, in 128-blocks
SLOT_BASE = [12, 8, 4, 0]   # head (0-indexed) = SLOT_BASE[s] + (core % 4)

F32 = mybir.dt.float32
BF16 = mybir.dt.bfloat16

EXP_BIAS = -12.0
BF16NP = ml_dtypes.bfloat16


def window_strips(s, t):
    """Strips (s, J, i0, i1) of window t for slot s (banded causal)."""
    B = BANDS[s]
    res = []
    for J in range(max(0, 4 * t - B + 1), 4 * t + 4):
        i0 = max(TSB * t, P * J)
        i1 = min(TSB * t + TSB, P * (J + B), S)
        if i1 > i0:
            res.append((s, J, i0, i1))
    return res


def group_strips(strips_w, cap=1024):
    groups, cur, w = [], [], 0
    for (s, J, i0, i1) in strips_w:
        if w + (i1 - i0) > cap and cur:
            groups.append(cur)
            cur, w = [], 0
        cur.append((s, J, i0, i1))
        w += i1 - i0
    if cur:
        groups.append(cur)
    return groups


def build_kernel():
    nc = bacc.Bacc("TRN2")

    xq = nc.dram_tensor("xq", [D, S], BF16, kind="ExternalInput")
    xk = nc.dram_tensor("xk", [D, S], BF16, kind="ExternalInput")
    xv = nc.dram_tensor("xv", [D, S], BF16, kind="ExternalInput")
    wq = nc.dram_tensor("wq", [D, CW], BF16, kind="ExternalInput")
    wk = nc.dram_tensor("wk", [D, CW], BF16, kind="ExternalInput")
    wv = nc.dram_tensor("wv", [D, CW], BF16, kind="ExternalInput")
    augq = nc.dram_tensor("augq", [HPC, AUG, S], BF16, kind="ExternalInput")
    augk = nc.dram_tensor("augk", [AUG, S], BF16, kind="ExternalInput")
    tri = nc.dram_tensor("tri", [P, P], BF16, kind="ExternalInput")
    out = nc.dram_tensor("out", [S, CW], F32, kind="ExternalOutput")

    x_drams = [xq, xk, xv]
    w_drams = [wq, wk, wv]

    with tile.TileContext(nc) as tc:
        with (
            tc.tile_pool(name="cn", bufs=1) as cnp,
            tc.tile_pool(name="wp", bufs=1) as wp,
            tc.tile_pool(name="qk", bufs=1) as qkp,
            tc.tile_pool(name="vv", bufs=1) as vvp,
            tc.tile_pool(name="xp", bufs=3) as xp,
            tc.tile_pool(name="pt", bufs=16) as ptp,
            tc.tile_pool(name="rc", bufs=3) as rcp,
            tc.tile_pool(name="ob", bufs=1) as obp,
            tc.tile_pool(name="pq", bufs=2, space="PSUM") as pqp,
            tc.tile_pool(name="sc", bufs=2, space="PSUM") as scp,
            tc.tile_pool(name="av", bufs=2, space="PSUM") as avp,
        ):
            # ---- startup DMAs in critical-path order: (wv, xv0), (wq, xq0),
            # slot-0/1 augs, (wk, xk0), slot-2/3 augs — V/Q chains and the
            # first windows (slots 0/1) start as soon as their data lands ----
            def emit_xdma_one(xt, t, xi):
                nc.sync.dma_start(
                    xt[:, :, xi, :],
                    x_drams[xi].rearrange("(ko p) t -> p ko t", p=P)[
                        :, :, t * TSB:(t + 1) * TSB],
                )

            def emit_xdma(t):
                xt = xp.tile([P, NKC, 3, TSB], BF16, tag="x", name=f"x{t}")
                for xi in range(3):
                    emit_xdma_one(xt, t, xi)
                return xt

            w_tiles = [None, None, None]
            xt0 = xp.tile([P, NKC, 3, TSB], BF16, tag="x", name="x0")

            def emit_wdma(i):
                wt = wp.tile([P, NKC, CW], BF16, tag=f"w{i}", name=f"w{i}")
                nc.sync.dma_start(
                    wt[:], w_drams[i].rearrange("(ko p) c -> p ko c", p=P))
                w_tiles[i] = wt

            emit_wdma(2)
            emit_xdma_one(xt0, 0, 2)
            emit_wdma(0)
            emit_xdma_one(xt0, 0, 0)

            # ---- constants ----
            tri_t = cnp.tile([P, P], BF16, tag="tri", name="tri_t")
            nc.sync.dma_start(tri_t[:], tri[:])
            ebias = cnp.tile([P, 1], F32, tag="ebias", name="ebias")
            nc.gpsimd.memset(ebias[:], EXP_BIAS)

            # ---- per-slot Q~^T / K~^T [67, S] bf16; V~ [128, NJ, HPC, 65] ----
            qs = [qkp.tile([QROWS, S], BF16, tag=f"q{s}", name=f"q{s}")
                  for s in range(HPC)]
            ks = [qkp.tile([QROWS, S], BF16, tag=f"k{s}", name=f"k{s}")
                  for s in range(HPC)]
            # V~ sub-block = 72 cols: V data 0:64, then a 16-byte aligned
            # constant lane [64:72) holding the softmax-denominator ones at
            # col VONE. Padding keeps any >=4-byte RMW write granule away
            # from the bf16 V data (same-engine memsets, ordered before the
            # DVE V copies).
            v_t = vvp.tile([P, NJ, HPC, VSTR], BF16, tag="v", name="v_t")
            nc.vector.memset(v_t[:, :, :, HWID:VSTR], 0.0)
            nc.vector.memset(v_t[:, :, :, VONE:VONE + 1], 1.0)

            # slot-0/1 aug rows early (first windows), then (wk, xk0),
            # then slot-2/3 augs
            for s in range(2):
                nc.sync.dma_start(qs[s][HWID:QROWS, :], augq[s])
                nc.sync.dma_start(ks[s][HWID:QROWS, :], augk[:])
            emit_wdma(1)
            emit_xdma_one(xt0, 0, 1)
            for s in range(2, HPC):
                nc.sync.dma_start(qs[s][HWID:QROWS, :], augq[s])
                nc.sync.dma_start(ks[s][HWID:QROWS, :], augk[:])

            # ---- output staging [P, NJ, CW] f32 ----
            out_sb = obp.tile([P, NJ, CW], F32, tag="ob", name="out_sb")

            # ---------- emission helpers ----------
            def proj_chain_units(t, xt):
                units = []
                if t == 0:
                    # V first (xv lands first), then Q/K cc0 (slots 0/1 ->
                    # first windows), then cc1
                    for u in range(TSB // P):
                        units.append(("v", t, xt, u))
                    for cc in range(2):
                        for pi in range(2):
                            units.append(("qk", t, xt, pi, cc))
                    return units
                for pi in range(2):
                    for cc in range(2):
                        units.append(("qk", t, xt, pi, cc))
                for u in range(TSB // P):
                    units.append(("v", t, xt, u))
                return units

            def emit_chain(unit):
                kind = unit[0]
                if kind == "qk":
                    _, t, xt, pi, cc = unit
                    dsts = qs if pi == 0 else ks
                    ps = pqp.tile([P, TSB], F32, tag="pq",
                                  name=f"pq{t}_{pi}_{cc}")
                    for kk in range(NKC):
                        nc.tensor.matmul(
                            ps[:],
                            lhsT=w_tiles[pi][:, kk, cc * P:(cc + 1) * P],
                            rhs=xt[:, kk, pi, :],
                            start=(kk == 0),
                            stop=(kk == NKC - 1),
                        )
                    nc.vector.tensor_copy(
                        dsts[2 * cc][0:HWID, t * TSB:(t + 1) * TSB],
                        ps[0:HWID, :],
                    )
                    nc.vector.tensor_copy(
                        dsts[2 * cc + 1][0:HWID, t * TSB:(t + 1) * TSB],
                        ps[HWID:P, :],
                    )
                else:
                    _, t, xt, u = unit
                    tt = t * (TSB // P) + u
                    ps = pqp.tile([P, TSB], F32, tag="pq", name=f"pv{t}_{u}")
                    for kk in range(NKC):
                        nc.tensor.matmul(
                            ps[:, 0:CW],
                            lhsT=xt[:, kk, 2, u * P:(u + 1) * P],
                            rhs=w_tiles[2][:, kk, :],
                            start=(kk == 0),
                            stop=(kk == NKC - 1),
                        )
                    nc.vector.tensor_copy(
                        v_t[:, tt, :, 0:HWID],
                        ps[:, 0:CW].rearrange("p (h w) -> p h w", h=HPC),
                    )

            def emit_scores_exp(t, g, gi):
                width = sum(i1 - i0 for (_, _, i0, i1) in g)
                sc = scp.tile([P, 1024], F32, tag="sc", name=f"sc{t}_{gi}")
                o = 0
                offs = []
                for (s, J, i0, i1) in g:
                    W = i1 - i0
                    a = 0
                    while a < W:  # split at psum bank boundaries (512 cols)
                        b = min(W, a + 512 - (o + a) % 512)
                        nc.tensor.matmul(
                            sc[:, o + a:o + b],
                            lhsT=ks[s][0:QROWS, P * J:P * (J + 1)],
                            rhs=qs[s][0:QROWS, i0 + a:i0 + b],
                            start=True,
                            stop=True,
                        )
                        a = b
                    offs.append((s, J, i0, i1, o))
                    o += W
                pt = ptp.tile([P, 1024], BF16, tag="pt", name=f"pt{t}_{gi}")
                nc.scalar.activation(
                    pt[:, 0:width], sc[:, 0:width],
                    mybir.ActivationFunctionType.Exp,
                    bias=ebias[:], scale=1.0,
                )
                for (s, J, i0, i1, off) in offs:
                    if i0 == P * J:
                        # mask the diagonal 128-strip (keep j <= i), on Pool
                        nc.gpsimd.tensor_mul(
                            pt[:, off:off + P], pt[:, off:off + P], tri_t[:]
                        )
                return pt, offs

            def emit_epilogue(s, t, acc):
                rec = rcp.tile([P, 4, 1], F32, tag="rc", name=f"rc{s}_{t}")
                nc.vector.reciprocal(rec[:], acc[:, :, VONE:VONE + 1])
                for r in range(4):
                    nc.vector.tensor_scalar_mul(
                        out_sb[:, 4 * t + r, s * HWID:(s + 1) * HWID],
                        acc[:, r, 0:HWID],
                        rec[:, r, :],
                    )
                if s % 2 == 1 and t in (NTSB // 2 - 1, NTSB - 1):
                    # slot pair: 128 contiguous f32 columns -> 512B elems;
                    # rows [0,1024) leave early (after t=1), only the upper
                    # half remains in the tail
                    a0 = 0 if t == NTSB // 2 - 1 else NJ // 2
                    a1 = NJ // 2 if t == NTSB // 2 - 1 else NJ
                    nc.sync.dma_start(
                        out.rearrange("(a p) c -> p a c", p=P)[
                            :, a0:a1, (s - 1) * HWID:(s + 1) * HWID],
                        out_sb[:, a0:a1, (s - 1) * HWID:(s + 1) * HWID],
                    )

            # pending completed windows: list of [s, t, jmap, age]
            pends = []

            def flush_window():
                if not pends:
                    return
                s, t, jmap, _ = pends.pop(0)
                B = BANDS[s]
                acc = avp.tile([P, 4, 80], F32, tag="acc", name=f"acc{s}_{t}")
                # one fully-closed accumulation chain per i-block (PSUM bank
                # allows only one open chain at a time)
                for r in range(4):
                    i_blk = 4 * t + r
                    jst = max(0, i_blk - B + 1)
                    for J in range(jst, i_blk + 1):
                        pt, off, i0 = jmap[J]
                        col = off + i_blk * P - i0
                        nc.tensor.matmul(
                            acc[:, r, 0:VONE + 1],
                            lhsT=pt[:, col:col + P],
                            rhs=v_t[:, J, s, 0:VONE + 1],
                            start=(J == jst),
                            stop=(J == i_blk),
                        )
                emit_epilogue(s, t, acc)

            # ---------- main schedule ----------
            for unit in proj_chain_units(0, xt0):
                emit_chain(unit)

            for t in range(NTSB):
                if t + 1 < NTSB:
                    xt_next = emit_xdma(t + 1)
                    next_chains = proj_chain_units(t + 1, xt_next)
                else:
                    next_chains = []
                all_strips = []
                last_of = {}
                for s in range(HPC):
                    ws = window_strips(s, t)
                    all_strips.extend(ws)
                    last_of[ws[-1]] = s
                groups = group_strips(all_strips)
                total_groups = len(groups)
                ci = 0
                jmaps = {s: {} for s in range(HPC)}
                for gi, g in enumerate(groups):
                    pt, offs = emit_scores_exp(t, g, gi)
                    for (s, J, i0, i1, off) in offs:
                        jmaps[s][J] = (pt, off, i0)
                    for pw in pends:
                        pw[3] += 1
                    for strip in g:
                        if strip in last_of:
                            pends.append([last_of[strip], t, jmaps[strip[0]], 0])
                    while (ci < len(next_chains)
                           and ci < ((gi + 1) * len(next_chains))
                           // total_groups):
                        emit_chain(next_chains[ci])
                        ci += 1
                    while pends and (pends[0][3] >= 3 or len(pends) > 1):
                        flush_window()
                while ci < len(next_chains):
                    emit_chain(next_chains[ci])
                    ci += 1
            while pends:
                flush_window()

    nc.compile()
    return nc


_NC = None


def _get_nc():
    global _NC
    if _NC is None:
        _NC = build_kernel()
    return _NC


def kernel(queries, keys, values, mask, Wq, Wk, Wv):
    B = queries.shape[0]
    asc = np.ascontiguousarray
    scale = 1.0 / np.sqrt(HWID)

    WqT = asc((np.asarray(Wq).T * scale).astype(np.float32)).astype(BF16NP)
    WkT = asc(np.asarray(Wk).T.astype(np.float32)).astype(BF16NP)
    WvT = asc(np.asarray(Wv).T.astype(np.float32)).astype(BF16NP)
    qTs = [asc(np.asarray(queries[b]).T.astype(np.float32)).astype(BF16NP)
           for b in range(B)]
    kTs = [asc(np.asarray(keys[b]).T.astype(np.float32)).astype(BF16NP)
           for b in range(B)]
    vTs = [asc(np.asarray(values[b]).T.astype(np.float32)).astype(BF16NP)
           for b in range(B)]

    slopes = (2.0 ** (-np.arange(1, H + 1) * (8.0 / H))).astype(np.float32)
    slopes_bf = slopes.astype(BF16NP).astype(np.float32)
    iv = np.arange(S, dtype=np.float32)
    j_hi = (np.arange(S) // 256 * 256).astype(np.float32)
    j_lo = (np.arange(S) % 256).astype(np.float32)
    augk_np = np.stack([j_hi, j_lo, np.ones(S, np.float32)]).astype(BF16NP)
    tri_np = np.asarray(
        np.arange(P)[:, None] <= np.arange(P)[None, :], dtype=np.float32
    ).astype(BF16NP)  # keep j<=i: rows p (j), cols u (i)

    nc = _get_nc()
    in_maps = []
    for c in range(8):
        b, k = divmod(c, 4)
        heads = [SLOT_BASE[s] + k for s in range(HPC)]
        aq = np.zeros((HPC, AUG, S), np.float32)
        for s, h in enumerate(heads):
            aq[s, 0, :] = slopes_bf[h]
            aq[s, 1, :] = slopes_bf[h]
            aq[s, 2, :] = -slopes_bf[h] * iv
        cols = np.concatenate(
            [np.arange(h * HWID, (h + 1) * HWID) for h in heads])
        in_maps.append({
            "xq": qTs[b], "xk": kTs[b], "xv": vTs[b],
            "wq": asc(WqT[:, cols]), "wk": asc(WkT[:, cols]),
            "wv": asc(WvT[:, cols]),
            "augq": aq.astype(BF16NP), "augk": augk_np,
            "tri": tri_np,
        })

    res = run_bass_kernel_spmd(nc, in_maps, core_ids=list(range(8)))
    outp = np.empty((B, S, D), np.float32)
    for c in range(8):
        b, k = divmod(c, 4)
        for s in range(HPC):
            h = SLOT_BASE[s] + k
            outp[b, :, h * HWID:(h + 1) * HWID] = \
                res.results[c]["out"][:, s * HWID:(s + 1) * HWID]
    return outp


# revision 60
# speedup vs baseline: 1.1639x; 1.0843x over previous
"""MHSA (dense transformer, ALiBi + causal) TRN2 Bass kernel, 8-core SPMD.

v4 design:
- Sharding: batch (2) x head-quartile -> 8 cores, heads REBALANCED so every
  core gets one head from each ALiBi-slope quartile: core c (b=c//4, k=c%4)
  computes heads [12+k, 8+k, 4+k, 0+k] (0-indexed) of batch b. Slot s on all
  cores then shares one causal band -> SPMD-friendly block skipping.
- Banded causal attention: ALiBi slope*(i-j) >~ 32 => weight < e^-32,
  skipped structurally. Bands (in 128-blocks) per slot: [16, 16, 5, 2].
- All matmuls bf16 (1 cyc/row on PE at any width). ALiBi folded into 3
  bf16-exact aug contraction rows:
    Q~ = [Q; slope; slope; -slope*i],  K~ = [K; j_hi; j_lo; 1]
  with j_hi multiple of 256 and j_lo in [0,256) - both exact in bf16; the
  -slope*i row is a per-row shift that cancels in softmax.
- Projection phase (x/w bf16, PSUM f32): Q^T/K^T copied to bf16 SBUF slot
  tiles by DVE, V by DVE into a [j, slot, 65] bf16 tile with a ones column
  (column 64 of the AV output becomes the softmax denominator).
- Attention interleaved with projections by i-window: window t = i in
  [512t, 512t+512), its strip groups round-robined with the projection
  chains of superblock t+1 so ACT exp overlaps PE projection matmuls.
- Scores S^T[j,i] per (slot, J): strips grouped into <=1024-col PSUM tiles,
  ONE exp (ACT) per group -> bf16 P^T; diagonal 128-strips masked on Pool
  (gpsimd) with a 0/1 triangle.
- AV: out[i(128), 65] accumulated over J into a [128, 4, 80] PSUM tile.
  IMPORTANT: PSUM accumulation groups are tracked per 2KB bank - only ONE
  chain may be open per bank at a time (interleaved starts lazily re-zero
  the bank). So all AV chains of a window are emitted at window END, one
  i-block at a time, each chain fully closed before the next starts.
- Epilogue = batched reciprocal + tensor_scalar multiplies (DVE) into an
  SBUF staging tile, DMA'd out per slot-pair (512B elements).
"""

import numpy as np
import ml_dtypes

import concourse.bass as bass
import concourse.mybir as mybir
import concourse.tile as tile
from concourse import bacc
from concourse.bass_utils import run_bass_kernel_spmd

P = 128
S = 2048
D = 1024
H = 16
HWID = 64
HPC = 4           # head slots per core
CW = HPC * HWID   # 256
NKC = D // P      # 8 contraction chunks
NTSB = 4          # t super-blocks (projection + attention i-windows)
TSB = S // NTSB   # 512
NJ = S // P       # 16
AUG = 3
QROWS = HWID + AUG  # 67
VSTR = 72           # V~ sub-block stride: 64 data + aligned ones lane
VONE = 64           # ones column (softmax denominator), first col of the lane
BANDS = [16, 16, 5, 2]      # causal band per slot, in 128-blocks
SLOT_BASE = [12, 8, 4, 0]   # head (0-indexed) = SLOT_BASE[s] + (core % 4)

F32 = mybir.dt.float32
BF16 = mybir.dt.bfloat16

EXP_BIAS = -12.0
BF16NP = ml_dtypes.bfloat16


def window_strips(s, t):
    """Strips (s, J, i0, i1) of window t for slot s (banded causal)."""
    B = BANDS[s]
    res = []
    for J in range(max(0, 4 * t - B + 1), 4 * t + 4):
        i0 = max(TSB * t, P * J)
        i1 = min(TSB * t + TSB, P * (J + B), S)
        if i1 > i0:
            res.append((s, J, i0, i1))
    return res


def group_strips(strips_w, cap=1024):
    groups, cur, w = [], [], 0
    for (s, J, i0, i1) in strips_w:
        if w + (i1 - i0) > cap and cur:
            groups.append(cur)
            cur, w = [], 0
        cur.append((s, J, i0, i1))
        w += i1 - i0
    if cur:
        groups.append(cur)
    return groups


def build_kernel():
    nc = bacc.Bacc("TRN2")

    xq = nc.dram_tensor("xq", [D, S], BF16, kind="ExternalInput")
    xk = nc.dram_tensor("xk", [D, S], BF16, kind="ExternalInput")
    xv = nc.dram_tensor("xv", [D, S], BF16, kind="ExternalInput")
    wq = nc.dram_tensor("wq", [D, CW], BF16, kind="ExternalInput")
    wk = nc.dram_tensor("wk", [D, CW], BF16, kind="ExternalInput")
    wv = nc.dram_tensor("wv", [D, CW], BF16, kind="ExternalInput")
    augq = nc.dram_tensor("augq", [HPC, AUG, S], BF16, kind="ExternalInput")
    augk = nc.dram_tensor("augk", [AUG, S], BF16, kind="ExternalInput")
    tri = nc.dram_tensor("tri", [P, P], BF16, kind="ExternalInput")
    out = nc.dram_tensor("out", [S, CW], F32, kind="ExternalOutput")

    x_drams = [xq, xk, xv]
    w_drams = [wq, wk, wv]

    with tile.TileContext(nc) as tc:
        with (
            tc.tile_pool(name="cn", bufs=1) as cnp,
            tc.tile_pool(name="wp", bufs=1) as wp,
            tc.tile_pool(name="qk", bufs=1) as qkp,
            tc.tile_pool(name="vv", bufs=1) as vvp,
            tc.tile_pool(name="xp", bufs=3) as xp,
            tc.tile_pool(name="pt", bufs=16) as ptp,
            tc.tile_pool(name="rc", bufs=3) as rcp,
            tc.tile_pool(name="ob", bufs=1) as obp,
            tc.tile_pool(name="pq", bufs=2, space="PSUM") as pqp,
            tc.tile_pool(name="sc", bufs=2, space="PSUM") as scp,
            tc.tile_pool(name="av", bufs=2, space="PSUM") as avp,
        ):
            # ---- startup DMAs in critical-path order: (wv, xv0), (wq, xq0),
            # slot-0/1 augs, (wk, xk0), slot-2/3 augs — V/Q chains and the
            # first windows (slots 0/1) start as soon as their data lands ----
            def emit_xdma_one(xt, t, xi):
                nc.sync.dma_start(
                    xt[:, :, xi, :],
                    x_drams[xi].rearrange("(ko p) t -> p ko t", p=P)[
                        :, :, t * TSB:(t + 1) * TSB],
                )

            def emit_xdma(t):
                xt = xp.tile([P, NKC, 3, TSB], BF16, tag="x", name=f"x{t}")
                for xi in range(3):
                    emit_xdma_one(xt, t, xi)
                return xt

            w_tiles = [None, None, None]
            xt0 = xp.tile([P, NKC, 3, TSB], BF16, tag="x", name="x0")

            def emit_wdma(i):
                wt = wp.tile([P, NKC, CW], BF16, tag=f"w{i}", name=f"w{i}")
                nc.sync.dma_start(
                    wt[:], w_drams[i].rearrange("(ko p) c -> p ko c", p=P))
                w_tiles[i] = wt

            emit_wdma(2)
            # xv in halves: V chains u=0,1 need only cols 0:256
            nc.sync.dma_start(
                xt0[:, :, 2, 0:256],
                x_drams[2].rearrange("(ko p) t -> p ko t", p=P)[:, :, 0:256])
            nc.sync.dma_start(
                xt0[:, :, 2, 256:512],
                x_drams[2].rearrange("(ko p) t -> p ko t", p=P)[:, :, 256:512])
            emit_wdma(0)
            nc.sync.dma_start(
                xt0[:, :, 0, 0:256],
                x_drams[0].rearrange("(ko p) t -> p ko t", p=P)[:, :, 0:256])
            nc.sync.dma_start(
                xt0[:, :, 0, 256:512],
                x_drams[0].rearrange("(ko p) t -> p ko t", p=P)[:, :, 256:512])

            # ---- constants ----
            tri_t = cnp.tile([P, P], BF16, tag="tri", name="tri_t")
            nc.sync.dma_start(tri_t[:], tri[:])
            ebias = cnp.tile([P, 1], F32, tag="ebias", name="ebias")
            nc.gpsimd.memset(ebias[:], EXP_BIAS)

            # ---- per-slot Q~^T / K~^T [67, S] bf16; V~ [128, NJ, HPC, 65] ----
            qs = [qkp.tile([QROWS, S], BF16, tag=f"q{s}", name=f"q{s}")
                  for s in range(HPC)]
            ks = [qkp.tile([QROWS, S], BF16, tag=f"k{s}", name=f"k{s}")
                  for s in range(HPC)]
            # V~ sub-block = 72 cols: V data 0:64, then a 16-byte aligned
            # constant lane [64:72) holding the softmax-denominator ones at
            # col VONE. Padding keeps any >=4-byte RMW write granule away
            # from the bf16 V data (same-engine memsets, ordered before the
            # DVE V copies).
            v_t = vvp.tile([P, NJ, HPC, VSTR], BF16, tag="v", name="v_t")
            nc.vector.memset(v_t[:, :, :, HWID:VSTR], 0.0)
            nc.vector.memset(v_t[:, :, :, VONE:VONE + 1], 1.0)

            # slot-0/1 aug rows early (first windows), then (wk, xk0),
            # then slot-2/3 augs
            for s in range(2):
                nc.sync.dma_start(qs[s][HWID:QROWS, :], augq[s])
                nc.sync.dma_start(ks[s][HWID:QROWS, :], augk[:])
            emit_wdma(1)
            emit_xdma_one(xt0, 0, 1)
            for s in range(2, HPC):
                nc.sync.dma_start(qs[s][HWID:QROWS, :], augq[s])
                nc.sync.dma_start(ks[s][HWID:QROWS, :], augk[:])

            # ---- output staging [P, NJ, CW] f32 ----
            out_sb = obp.tile([P, NJ, CW], F32, tag="ob", name="out_sb")

            # ---------- emission helpers ----------
            def cc_chain_units(t, xt, cc):
                return [("qk", t, xt, pi, cc) for pi in range(2)]

            def v_chain_units(t, xt):
                return [("v", t, xt, u) for u in range(TSB // P)]

            def emit_chain(unit):
                kind = unit[0]
                if kind == "qk":
                    _, t, xt, pi, cc = unit
                    dsts = qs if pi == 0 else ks
                    ps = pqp.tile([P, TSB], F32, tag="pq",
                                  name=f"pq{t}_{pi}_{cc}")
                    # t0 Q cc0: two sequential half-chains so the first
                    # starts after the half x-DMA lands
                    halves = ([(0, 256), (256, TSB)]
                              if (t == 0 and pi == 0 and cc == 0)
                              else [(0, TSB)])
                    for (a, b) in halves:
                        for kk in range(NKC):
                            nc.tensor.matmul(
                                ps[:, a:b],
                                lhsT=w_tiles[pi][:, kk, cc * P:(cc + 1) * P],
                                rhs=xt[:, kk, pi, a:b],
                                start=(kk == 0),
                                stop=(kk == NKC - 1),
                            )
                    nc.vector.tensor_copy(
                        dsts[2 * cc][0:HWID, t * TSB:(t + 1) * TSB],
                        ps[0:HWID, :],
                    )
                    nc.vector.tensor_copy(
                        dsts[2 * cc + 1][0:HWID, t * TSB:(t + 1) * TSB],
                        ps[HWID:P, :],
                    )
                else:
                    _, t, xt, u = unit
                    tt = t * (TSB // P) + u
                    ps = pqp.tile([P, TSB], F32, tag="pq", name=f"pv{t}_{u}")
                    for kk in range(NKC):
                        nc.tensor.matmul(
                            ps[:, 0:CW],
                            lhsT=xt[:, kk, 2, u * P:(u + 1) * P],
                            rhs=w_tiles[2][:, kk, :],
                            start=(kk == 0),
                            stop=(kk == NKC - 1),
                        )
                    nc.vector.tensor_copy(
                        v_t[:, tt, :, 0:HWID],
                        ps[:, 0:CW].rearrange("p (h w) -> p h w", h=HPC),
                    )

            def emit_scores_exp(t, g, gi):
                width = sum(i1 - i0 for (_, _, i0, i1) in g)
                sc = scp.tile([P, 1024], F32, tag="sc", name=f"sc{t}_{gi}")
                o = 0
                offs = []
                for (s, J, i0, i1) in g:
                    W = i1 - i0
                    a = 0
                    while a < W:  # split at psum bank boundaries (512 cols)
                        b = min(W, a + 512 - (o + a) % 512)
                        nc.tensor.matmul(
                            sc[:, o + a:o + b],
                            lhsT=ks[s][0:QROWS, P * J:P * (J + 1)],
                            rhs=qs[s][0:QROWS, i0 + a:i0 + b],
                            start=True,
                            stop=True,
                        )
                        a = b
                    offs.append((s, J, i0, i1, o))
                    o += W
                pt = ptp.tile([P, 1024], BF16, tag="pt", name=f"pt{t}_{gi}")
                nc.scalar.activation(
                    pt[:, 0:width], sc[:, 0:width],
                    mybir.ActivationFunctionType.Exp,
                    bias=ebias[:], scale=1.0,
                )
                for (s, J, i0, i1, off) in offs:
                    if i0 == P * J:
                        # mask the diagonal 128-strip (keep j <= i), on Pool
                        nc.gpsimd.tensor_mul(
                            pt[:, off:off + P], pt[:, off:off + P], tri_t[:]
                        )
                return pt, offs

            def emit_epilogue(s, t, acc):
                rec = rcp.tile([P, 4, 1], F32, tag="rc", name=f"rc{s}_{t}")
                nc.vector.reciprocal(rec[:], acc[:, :, VONE:VONE + 1])
                for r in range(4):
                    nc.vector.tensor_scalar_mul(
                        out_sb[:, 4 * t + r, s * HWID:(s + 1) * HWID],
                        acc[:, r, 0:HWID],
                        rec[:, r, :],
                    )
                if s % 2 == 1 and t >= 1:
                    # slot pair: 128 contiguous f32 columns -> 512B elems;
                    # finished row-blocks stream out per block, only rows
                    # [1536,2048) remain in the tail
                    a0, a1 = {1: (0, 8), 2: (8, 12), 3: (12, 16)}[t]
                    nc.sync.dma_start(
                        out.rearrange("(a p) c -> p a c", p=P)[
                            :, a0:a1, (s - 1) * HWID:(s + 1) * HWID],
                        out_sb[:, a0:a1, (s - 1) * HWID:(s + 1) * HWID],
                    )

            # pending completed windows: list of [s, t, jmap, age]
            pends = []
            vq = []       # current block's deferred V chains (unemitted)
            cur_t = [0]

            def flush_window():
                if not pends:
                    return
                if pends[0][1] == cur_t[0]:
                    # flushing a window of the CURRENT block: its AV reads
                    # v_t of this superblock - drain the deferred V chains
                    # first (they must precede the read in program order)
                    while vq:
                        emit_chain(vq.pop(0))
                s, t, jmap, _ = pends.pop(0)
                B = BANDS[s]
                acc = avp.tile([P, 4, 80], F32, tag="acc", name=f"acc{s}_{t}")
                # one fully-closed accumulation chain per i-block (PSUM bank
                # allows only one open chain at a time)
                for r in range(4):
                    i_blk = 4 * t + r
                    jst = max(0, i_blk - B + 1)
                    for J in range(jst, i_blk + 1):
                        pt, off, i0 = jmap[J]
                        col = off + i_blk * P - i0
                        nc.tensor.matmul(
                            acc[:, r, 0:VONE + 1],
                            lhsT=pt[:, col:col + P],
                            rhs=v_t[:, J, s, 0:VONE + 1],
                            start=(J == jst),
                            stop=(J == i_blk),
                        )
                emit_epilogue(s, t, acc)

            # ---------- main schedule ----------
            # upfront: V(0) (xv lands first) then Q/K cc0(0) - just enough
            # for the first windows (slots 0/1). Everything else defers into
            # the attention blocks as PE filler, latest-needed last:
            # block t = [cc1(t): its s2/s3 scores] + [V(t): its AV flushes]
            #         + [cc0(t+1): next block's s0/s1 scores]
            for unit in v_chain_units(0, xt0) + cc_chain_units(0, xt0, 0):
                emit_chain(unit)

            xts = {0: xt0}

            for t in range(NTSB):
                cur_t[0] = t
                nv = 0
                if t > 0:
                    vq.extend(v_chain_units(t, xts[t]))
                    nv = len(vq)
                cc1q = cc_chain_units(t, xts[t], 1)
                ncc1 = len(cc1q)
                next_chains = []
                if t + 1 < NTSB:
                    xt_next = emit_xdma(t + 1)
                    xts[t + 1] = xt_next
                    next_chains = cc_chain_units(t + 1, xt_next, 0)
                all_strips = []
                last_of = {}
                for s in range(HPC):
                    ws = window_strips(s, t)
                    all_strips.extend(ws)
                    last_of[ws[-1]] = s
                if t == NTSB - 1:
                    # split the final slot's groups so the last serial exp
                    # in the tail is short
                    head = [st for st in all_strips if st[0] < 3]
                    tl3 = [st for st in all_strips if st[0] == 3]
                    groups = group_strips(head) + group_strips(tl3, cap=512)
                else:
                    groups = group_strips(all_strips)
                total_groups = len(groups)
                first_s2 = next((i for i, g in enumerate(groups)
                                 if any(st[0] >= 2 for st in g)),
                                total_groups)

                ci = 0
                jmaps = {s: {} for s in range(HPC)}
                for gi, g in enumerate(groups):
                    if cc1q and any(st[0] >= 2 for st in g):
                        # this group's scores read qs/ks of slots 2/3:
                        # their projection chains must precede it
                        while cc1q:
                            emit_chain(cc1q.pop(0))
                    pt, offs = emit_scores_exp(t, g, gi)
                    for (s, J, i0, i1, off) in offs:
                        jmaps[s][J] = (pt, off, i0)
                    for pw in pends:
                        pw[3] += 1
                    for strip in g:
                        if strip in last_of:
                            pends.append([last_of[strip], t, jmaps[strip[0]], 0])
                    # V chains spread over the first half of the block,
                    # cc1 before the first s2/s3 scores
                    while vq and nv - len(vq) < ((gi + 1) * nv * 2) \
                            // total_groups:
                        emit_chain(vq.pop(0))
                    while cc1q and ncc1 - len(cc1q) < \
                            ((gi + 1) * ncc1) // max(1, first_s2):
                        emit_chain(cc1q.pop(0))
                    while (ci < len(next_chains)
                           and ci < ((gi + 1) * len(next_chains))
                           // total_groups):
                        emit_chain(next_chains[ci])
                        ci += 1
                    while pends and (pends[0][3] >= 11 or len(pends) > 3):
                        flush_window()
                while vq:
                    emit_chain(vq.pop(0))
                while cc1q:
                    emit_chain(cc1q.pop(0))
                while ci < len(next_chains):
                    emit_chain(next_chains[ci])
                    ci += 1
            while pends:
                flush_window()

    nc.compile()
    return nc


_NC = None


def _get_nc():
    global _NC
    if _NC is None:
        _NC = build_kernel()
    return _NC


def kernel(queries, keys, values, mask, Wq, Wk, Wv):
    B = queries.shape[0]
    asc = np.ascontiguousarray
    scale = 1.0 / np.sqrt(HWID)

    WqT = asc((np.asarray(Wq).T * scale).astype(np.float32)).astype(BF16NP)
    WkT = asc(np.asarray(Wk).T.astype(np.float32)).astype(BF16NP)
    WvT = asc(np.asarray(Wv).T.astype(np.float32)).astype(BF16NP)
    qTs = [asc(np.asarray(queries[b]).T.astype(np.float32)).astype(BF16NP)
           for b in range(B)]
    kTs = [asc(np.asarray(keys[b]).T.astype(np.float32)).astype(BF16NP)
           for b in range(B)]
    vTs = [asc(np.asarray(values[b]).T.astype(np.float32)).astype(BF16NP)
           for b in range(B)]

    slopes = (2.0 ** (-np.arange(1, H + 1) * (8.0 / H))).astype(np.float32)
    slopes_bf = slopes.astype(BF16NP).astype(np.float32)
    iv = np.arange(S, dtype=np.float32)
    j_hi = (np.arange(S) // 256 * 256).astype(np.float32)
    j_lo = (np.arange(S) % 256).astype(np.float32)
    augk_np = np.stack([j_hi, j_lo, np.ones(S, np.float32)]).astype(BF16NP)
    tri_np = np.asarray(
        np.arange(P)[:, None] <= np.arange(P)[None, :], dtype=np.float32
    ).astype(BF16NP)  # keep j<=i: rows p (j), cols u (i)

    nc = _get_nc()
    in_maps = []
    for c in range(8):
        b, k = divmod(c, 4)
        heads = [SLOT_BASE[s] + k for s in range(HPC)]
        aq = np.zeros((HPC, AUG, S), np.float32)
        for s, h in enumerate(heads):
            aq[s, 0, :] = slopes_bf[h]
            aq[s, 1, :] = slopes_bf[h]
            aq[s, 2, :] = -slopes_bf[h] * iv
        cols = np.concatenate(
            [np.arange(h * HWID, (h + 1) * HWID) for h in heads])
        in_maps.append({
            "xq": qTs[b], "xk": kTs[b], "xv": vTs[b],
            "wq": asc(WqT[:, cols]), "wk": asc(WkT[:, cols]),
            "wv": asc(WvT[:, cols]),
            "augq": aq.astype(BF16NP), "augk": augk_np,
            "tri": tri_np,
        })

    res = run_bass_kernel_spmd(nc, in_maps, core_ids=list(range(8)))
    outp = np.empty((B, S, D), np.float32)
    for c in range(8):
        b, k = divmod(c, 4)
        for s in range(HPC):
            h = SLOT_BASE[s] + k
            outp[b, :, h * HWID:(h + 1) * HWID] = \
                res.results[c]["out"][:, s * HWID:(s + 1) * HWID]
    return outp


# revision 62
# speedup vs baseline: 1.1672x; 1.0029x over previous
"""MHSA (dense transformer, ALiBi + causal) TRN2 Bass kernel, 8-core SPMD.

v4 design:
- Sharding: batch (2) x head-quartile -> 8 cores, heads REBALANCED so every
  core gets one head from each ALiBi-slope quartile: core c (b=c//4, k=c%4)
  computes heads [12+k, 8+k, 4+k, 0+k] (0-indexed) of batch b. Slot s on all
  cores then shares one causal band -> SPMD-friendly block skipping.
- Banded causal attention: ALiBi slope*(i-j) >~ 32 => weight < e^-32,
  skipped structurally. Bands (in 128-blocks) per slot: [16, 16, 5, 2].
- All matmuls bf16 (1 cyc/row on PE at any width). ALiBi folded into 3
  bf16-exact aug contraction rows:
    Q~ = [Q; slope; slope; -slope*i],  K~ = [K; j_hi; j_lo; 1]
  with j_hi multiple of 256 and j_lo in [0,256) - both exact in bf16; the
  -slope*i row is a per-row shift that cancels in softmax.
- Projection phase (x/w bf16, PSUM f32): Q^T/K^T copied to bf16 SBUF slot
  tiles by DVE, V by DVE into a [j, slot, 65] bf16 tile with a ones column
  (column 64 of the AV output becomes the softmax denominator).
- Attention interleaved with projections by i-window: window t = i in
  [512t, 512t+512), its strip groups round-robined with the projection
  chains of superblock t+1 so ACT exp overlaps PE projection matmuls.
- Scores S^T[j,i] per (slot, J): strips grouped into <=1024-col PSUM tiles,
  ONE exp (ACT) per group -> bf16 P^T; diagonal 128-strips masked on Pool
  (gpsimd) with a 0/1 triangle.
- AV: out[i(128), 65] accumulated over J into a [128, 4, 80] PSUM tile.
  IMPORTANT: PSUM accumulation groups are tracked per 2KB bank - only ONE
  chain may be open per bank at a time (interleaved starts lazily re-zero
  the bank). So all AV chains of a window are emitted at window END, one
  i-block at a time, each chain fully closed before the next starts.
- Epilogue = batched reciprocal + tensor_scalar multiplies (DVE) into an
  SBUF staging tile, DMA'd out per slot-pair (512B elements).
"""

import numpy as np
import ml_dtypes

import concourse.bass as bass
import concourse.mybir as mybir
import concourse.tile as tile
from concourse import bacc
from concourse.bass_utils import run_bass_kernel_spmd

P = 128
S = 2048
D = 1024
H = 16
HWID = 64
HPC = 4           # head slots per core
CW = HPC * HWID   # 256
NKC = D // P      # 8 contraction chunks
NTSB = 4          # t super-blocks (projection + attention i-windows)
TSB = S // NTSB   # 512
NJ = S // P       # 16
AUG = 3
QROWS = HWID + AUG  # 67
VSTR = 72           # V~ sub-block stride: 64 data + aligned ones lane
VONE = 64           # ones column (softmax denominator), first col of the lane
BANDS = [16, 16, 5, 2]      # causal band per slot, in 128-blocks
SLOT_BASE = [12, 8, 4, 0]   # head (0-indexed) = SLOT_BASE[s] + (core % 4)

F32 = mybir.dt.float32
BF16 = mybir.dt.bfloat16

EXP_BIAS = -12.0
BF16NP = ml_dtypes.bfloat16


def window_strips(s, t):
    """Strips (s, J, i0, i1) of window t for slot s (banded causal)."""
    B = BANDS[s]
    res = []
    for J in range(max(0, 4 * t - B + 1), 4 * t + 4):
        i0 = max(TSB * t, P * J)
        i1 = min(TSB * t + TSB, P * (J + B), S)
        if i1 > i0:
            res.append((s, J, i0, i1))
    return res


def group_strips(strips_w, cap=1024):
    groups, cur, w = [], [], 0
    for (s, J, i0, i1) in strips_w:
        if w + (i1 - i0) > cap and cur:
            groups.append(cur)
            cur, w = [], 0
        cur.append((s, J, i0, i1))
        w += i1 - i0
    if cur:
        groups.append(cur)
    return groups


def build_kernel():
    nc = bacc.Bacc("TRN2")

    xq = nc.dram_tensor("xq", [D, S], BF16, kind="ExternalInput")
    xk = nc.dram_tensor("xk", [D, S], BF16, kind="ExternalInput")
    xv = nc.dram_tensor("xv", [D, S], BF16, kind="ExternalInput")
    wq = nc.dram_tensor("wq", [D, CW], BF16, kind="ExternalInput")
    wk = nc.dram_tensor("wk", [D, CW], BF16, kind="ExternalInput")
    wv = nc.dram_tensor("wv", [D, CW], BF16, kind="ExternalInput")
    augq = nc.dram_tensor("augq", [HPC, AUG, S], BF16, kind="ExternalInput")
    augk = nc.dram_tensor("augk", [AUG, S], BF16, kind="ExternalInput")
    tri = nc.dram_tensor("tri", [P, P], BF16, kind="ExternalInput")
    out = nc.dram_tensor("out", [S, CW], F32, kind="ExternalOutput")

    x_drams = [xq, xk, xv]
    w_drams = [wq, wk, wv]

    with tile.TileContext(nc) as tc:
        with (
            tc.tile_pool(name="cn", bufs=1) as cnp,
            tc.tile_pool(name="wp", bufs=1) as wp,
            tc.tile_pool(name="qk", bufs=1) as qkp,
            tc.tile_pool(name="vv", bufs=1) as vvp,
            tc.tile_pool(name="xp", bufs=3) as xp,
            tc.tile_pool(name="pt", bufs=16) as ptp,
            tc.tile_pool(name="rc", bufs=3) as rcp,
            tc.tile_pool(name="ob", bufs=1) as obp,
            tc.tile_pool(name="pq", bufs=2, space="PSUM") as pqp,
            tc.tile_pool(name="sc", bufs=2, space="PSUM") as scp,
            tc.tile_pool(name="av", bufs=2, space="PSUM") as avp,
        ):
            # ---- startup DMAs in critical-path order: (wv, xv0), (wq, xq0),
            # slot-0/1 augs, (wk, xk0), slot-2/3 augs — V/Q chains and the
            # first windows (slots 0/1) start as soon as their data lands ----
            def emit_xdma_one(xt, t, xi):
                nc.sync.dma_start(
                    xt[:, :, xi, :],
                    x_drams[xi].rearrange("(ko p) t -> p ko t", p=P)[
                        :, :, t * TSB:(t + 1) * TSB],
                )

            def emit_xdma(t):
                xt = xp.tile([P, NKC, 3, TSB], BF16, tag="x", name=f"x{t}")
                for xi in range(3):
                    emit_xdma_one(xt, t, xi)
                return xt

            w_tiles = [None, None, None]
            xt0 = xp.tile([P, NKC, 3, TSB], BF16, tag="x", name="x0")

            def emit_wdma(i):
                wt = wp.tile([P, NKC, CW], BF16, tag=f"w{i}", name=f"w{i}")
                nc.sync.dma_start(
                    wt[:], w_drams[i].rearrange("(ko p) c -> p ko c", p=P))
                w_tiles[i] = wt

            emit_wdma(2)
            # xv in halves: V chains u=0,1 need only cols 0:256
            nc.sync.dma_start(
                xt0[:, :, 2, 0:256],
                x_drams[2].rearrange("(ko p) t -> p ko t", p=P)[:, :, 0:256])
            nc.sync.dma_start(
                xt0[:, :, 2, 256:512],
                x_drams[2].rearrange("(ko p) t -> p ko t", p=P)[:, :, 256:512])
            emit_wdma(0)
            nc.sync.dma_start(
                xt0[:, :, 0, 0:256],
                x_drams[0].rearrange("(ko p) t -> p ko t", p=P)[:, :, 0:256])
            nc.sync.dma_start(
                xt0[:, :, 0, 256:512],
                x_drams[0].rearrange("(ko p) t -> p ko t", p=P)[:, :, 256:512])

            # ---- constants ----
            tri_t = cnp.tile([P, P], BF16, tag="tri", name="tri_t")
            nc.sync.dma_start(tri_t[:], tri[:])
            ebias = cnp.tile([P, 1], F32, tag="ebias", name="ebias")
            nc.gpsimd.memset(ebias[:], EXP_BIAS)

            # ---- per-slot Q~^T / K~^T [67, S] bf16; V~ [128, NJ, HPC, 65] ----
            qs = [qkp.tile([QROWS, S], BF16, tag=f"q{s}", name=f"q{s}")
                  for s in range(HPC)]
            ks = [qkp.tile([QROWS, S], BF16, tag=f"k{s}", name=f"k{s}")
                  for s in range(HPC)]
            # V~ sub-block = 72 cols: V data 0:64, then a 16-byte aligned
            # constant lane [64:72) holding the softmax-denominator ones at
            # col VONE. Padding keeps any >=4-byte RMW write granule away
            # from the bf16 V data (same-engine memsets, ordered before the
            # DVE V copies).
            v_t = vvp.tile([P, NJ, HPC, VSTR], BF16, tag="v", name="v_t")
            nc.vector.memset(v_t[:, :, :, HWID:VSTR], 0.0)
            nc.vector.memset(v_t[:, :, :, VONE:VONE + 1], 1.0)

            # slot-0/1 aug rows early (first windows), then (wk, xk0),
            # then slot-2/3 augs
            for s in range(2):
                nc.sync.dma_start(qs[s][HWID:QROWS, :], augq[s])
                nc.sync.dma_start(ks[s][HWID:QROWS, :], augk[:])
            emit_wdma(1)
            emit_xdma_one(xt0, 0, 1)
            for s in range(2, HPC):
                nc.sync.dma_start(qs[s][HWID:QROWS, :], augq[s])
                nc.sync.dma_start(ks[s][HWID:QROWS, :], augk[:])

            # ---- output staging [P, NJ, CW] f32 ----
            out_sb = obp.tile([P, NJ, CW], F32, tag="ob", name="out_sb")

            # ---------- emission helpers ----------
            def cc_chain_units(t, xt, cc):
                return [("qk", t, xt, pi, cc) for pi in range(2)]

            def v_chain_units(t, xt):
                return [("v", t, xt, u) for u in range(TSB // P)]

            def emit_chain(unit):
                kind = unit[0]
                if kind == "qk":
                    _, t, xt, pi, cc = unit
                    dsts = qs if pi == 0 else ks
                    ps = pqp.tile([P, TSB], F32, tag="pq",
                                  name=f"pq{t}_{pi}_{cc}")
                    # t0 Q cc0: two sequential half-chains so the first
                    # starts after the half x-DMA lands
                    halves = ([(0, 256), (256, TSB)]
                              if (t == 0 and pi == 0 and cc == 0)
                              else [(0, TSB)])
                    for (a, b) in halves:
                        for kk in range(NKC):
                            nc.tensor.matmul(
                                ps[:, a:b],
                                lhsT=w_tiles[pi][:, kk, cc * P:(cc + 1) * P],
                                rhs=xt[:, kk, pi, a:b],
                                start=(kk == 0),
                                stop=(kk == NKC - 1),
                            )
                    nc.vector.tensor_copy(
                        dsts[2 * cc][0:HWID, t * TSB:(t + 1) * TSB],
                        ps[0:HWID, :],
                    )
                    nc.vector.tensor_copy(
                        dsts[2 * cc + 1][0:HWID, t * TSB:(t + 1) * TSB],
                        ps[HWID:P, :],
                    )
                else:
                    _, t, xt, u = unit
                    tt = t * (TSB // P) + u
                    ps = pqp.tile([P, TSB], F32, tag="pq", name=f"pv{t}_{u}")
                    for kk in range(NKC):
                        nc.tensor.matmul(
                            ps[:, 0:CW],
                            lhsT=xt[:, kk, 2, u * P:(u + 1) * P],
                            rhs=w_tiles[2][:, kk, :],
                            start=(kk == 0),
                            stop=(kk == NKC - 1),
                        )
                    nc.vector.tensor_copy(
                        v_t[:, tt, :, 0:HWID],
                        ps[:, 0:CW].rearrange("p (h w) -> p h w", h=HPC),
                    )

            def emit_scores_exp(t, g, gi):
                width = sum(i1 - i0 for (_, _, i0, i1) in g)
                sc = scp.tile([P, 1024], F32, tag="sc", name=f"sc{t}_{gi}")
                o = 0
                offs = []
                for (s, J, i0, i1) in g:
                    W = i1 - i0
                    a = 0
                    while a < W:  # split at psum bank boundaries (512 cols)
                        b = min(W, a + 512 - (o + a) % 512)
                        nc.tensor.matmul(
                            sc[:, o + a:o + b],
                            lhsT=ks[s][0:QROWS, P * J:P * (J + 1)],
                            rhs=qs[s][0:QROWS, i0 + a:i0 + b],
                            start=True,
                            stop=True,
                        )
                        a = b
                    offs.append((s, J, i0, i1, o))
                    o += W
                pt = ptp.tile([P, 1024], BF16, tag="pt", name=f"pt{t}_{gi}")
                nc.scalar.activation(
                    pt[:, 0:width], sc[:, 0:width],
                    mybir.ActivationFunctionType.Exp,
                    bias=ebias[:], scale=1.0,
                )
                for (s, J, i0, i1, off) in offs:
                    if i0 == P * J:
                        # mask the diagonal 128-strip (keep j <= i), on Pool
                        nc.gpsimd.tensor_mul(
                            pt[:, off:off + P], pt[:, off:off + P], tri_t[:]
                        )
                return pt, offs

            def emit_epilogue(s, t, acc):
                rec = rcp.tile([P, 4, 1], F32, tag="rc", name=f"rc{s}_{t}")
                nc.vector.reciprocal(rec[:], acc[:, :, VONE:VONE + 1])
                def pair_out(a0, a1):
                    nc.sync.dma_start(
                        out.rearrange("(a p) c -> p a c", p=P)[
                            :, a0:a1, (s - 1) * HWID:(s + 1) * HWID],
                        out_sb[:, a0:a1, (s - 1) * HWID:(s + 1) * HWID],
                    )

                last = s % 2 == 1 and t == NTSB - 1
                for r in range(4):
                    nc.vector.tensor_scalar_mul(
                        out_sb[:, 4 * t + r, s * HWID:(s + 1) * HWID],
                        acc[:, r, 0:HWID],
                        rec[:, r, :],
                    )
                    if last and r in (1, 3):
                        # tail rows stream out as their norms land
                        pair_out(*((12, 14) if r == 1 else (14, 16)))
                if s % 2 == 1 and t in (1, 2):
                    # finished row-blocks stream out per block
                    a0, a1 = (0, 8) if t == 1 else (8, 12)
                    pair_out(a0, a1)

            # pending completed windows: list of [s, t, jmap, age]
            pends = []
            vq = []       # current block's deferred V chains (unemitted)
            cur_t = [0]

            def flush_window():
                if not pends:
                    return
                if pends[0][1] == cur_t[0]:
                    # flushing a window of the CURRENT block: its AV reads
                    # v_t of this superblock - drain the deferred V chains
                    # first (they must precede the read in program order)
                    while vq:
                        emit_chain(vq.pop(0))
                s, t, jmap, _ = pends.pop(0)
                B = BANDS[s]
                acc = avp.tile([P, 4, 80], F32, tag="acc", name=f"acc{s}_{t}")
                # one fully-closed accumulation chain per i-block (PSUM bank
                # allows only one open chain at a time)
                for r in range(4):
                    i_blk = 4 * t + r
                    jst = max(0, i_blk - B + 1)
                    for J in range(jst, i_blk + 1):
                        pt, off, i0 = jmap[J]
                        col = off + i_blk * P - i0
                        nc.tensor.matmul(
                            acc[:, r, 0:VONE + 1],
                            lhsT=pt[:, col:col + P],
                            rhs=v_t[:, J, s, 0:VONE + 1],
                            start=(J == jst),
                            stop=(J == i_blk),
                        )
                emit_epilogue(s, t, acc)

            # ---------- main schedule ----------
            # upfront: V(0) (xv lands first) then Q/K cc0(0) - just enough
            # for the first windows (slots 0/1). Everything else defers into
            # the attention blocks as PE filler, latest-needed last:
            # block t = [cc1(t): its s2/s3 scores] + [V(t): its AV flushes]
            #         + [cc0(t+1): next block's s0/s1 scores]
            for unit in v_chain_units(0, xt0) + cc_chain_units(0, xt0, 0):
                emit_chain(unit)

            xts = {0: xt0}

            for t in range(NTSB):
                cur_t[0] = t
                nv = 0
                if t > 0:
                    vq.extend(v_chain_units(t, xts[t]))
                    nv = len(vq)
                cc1q = cc_chain_units(t, xts[t], 1)
                ncc1 = len(cc1q)
                next_chains = []
                if t + 1 < NTSB:
                    xt_next = emit_xdma(t + 1)
                    xts[t + 1] = xt_next
                    next_chains = cc_chain_units(t + 1, xt_next, 0)
                all_strips = []
                last_of = {}
                for s in range(HPC):
                    ws = window_strips(s, t)
                    all_strips.extend(ws)
                    last_of[ws[-1]] = s
                if t == NTSB - 1:
                    # split the final slot's groups so the last serial exp
                    # in the tail is short
                    head = [st for st in all_strips if st[0] < 3]
                    tl3 = [st for st in all_strips if st[0] == 3]
                    groups = group_strips(head) + group_strips(tl3, cap=512)
                else:
                    groups = group_strips(all_strips)
                total_groups = len(groups)
                first_s2 = next((i for i, g in enumerate(groups)
                                 if any(st[0] >= 2 for st in g)),
                                total_groups)

                ci = 0
                jmaps = {s: {} for s in range(HPC)}
                for gi, g in enumerate(groups):
                    if cc1q and any(st[0] >= 2 for st in g):
                        # this group's scores read qs/ks of slots 2/3:
                        # their projection chains must precede it
                        while cc1q:
                            emit_chain(cc1q.pop(0))
                    pt, offs = emit_scores_exp(t, g, gi)
                    for (s, J, i0, i1, off) in offs:
                        jmaps[s][J] = (pt, off, i0)
                    for pw in pends:
                        pw[3] += 1
                    for strip in g:
                        if strip in last_of:
                            pends.append([last_of[strip], t, jmaps[strip[0]], 0])
                    # V chains spread over the first half of the block,
                    # cc1 before the first s2/s3 scores
                    while vq and nv - len(vq) < ((gi + 1) * nv * 2) \
                            // total_groups:
                        emit_chain(vq.pop(0))
                    while cc1q and ncc1 - len(cc1q) < \
                            ((gi + 1) * ncc1) // max(1, first_s2):
                        emit_chain(cc1q.pop(0))
                    while (ci < len(next_chains)
                           and ci < ((gi + 1) * len(next_chains))
                           // total_groups):
                        emit_chain(next_chains[ci])
                        ci += 1
                    while pends and (pends[0][3] >= 11 or len(pends) > 3):
                        flush_window()
                while vq:
                    emit_chain(vq.pop(0))
                while cc1q:
                    emit_chain(cc1q.pop(0))
                while ci < len(next_chains):
                    emit_chain(next_chains[ci])
                    ci += 1
            while pends:
                flush_window()

    nc.compile()
    return nc


_NC = None


def _get_nc():
    global _NC
    if _NC is None:
        _NC = build_kernel()
    return _NC


def kernel(queries, keys, values, mask, Wq, Wk, Wv):
    B = queries.shape[0]
    asc = np.ascontiguousarray
    scale = 1.0 / np.sqrt(HWID)

    WqT = asc((np.asarray(Wq).T * scale).astype(np.float32)).astype(BF16NP)
    WkT = asc(np.asarray(Wk).T.astype(np.float32)).astype(BF16NP)
    WvT = asc(np.asarray(Wv).T.astype(np.float32)).astype(BF16NP)
    qTs = [asc(np.asarray(queries[b]).T.astype(np.float32)).astype(BF16NP)
           for b in range(B)]
    kTs = [asc(np.asarray(keys[b]).T.astype(np.float32)).astype(BF16NP)
           for b in range(B)]
    vTs = [asc(np.asarray(values[b]).T.astype(np.float32)).astype(BF16NP)
           for b in range(B)]

    slopes = (2.0 ** (-np.arange(1, H + 1) * (8.0 / H))).astype(np.float32)
    slopes_bf = slopes.astype(BF16NP).astype(np.float32)
    iv = np.arange(S, dtype=np.float32)
    j_hi = (np.arange(S) // 256 * 256).astype(np.float32)
    j_lo = (np.arange(S) % 256).astype(np.float32)
    augk_np = np.stack([j_hi, j_lo, np.ones(S, np.float32)]).astype(BF16NP)
    tri_np = np.asarray(
        np.arange(P)[:, None] <= np.arange(P)[None, :], dtype=np.float32
    ).astype(BF16NP)  # keep j<=i: rows p (j), cols u (i)

    nc = _get_nc()
    in_maps = []
    for c in range(8):
        b, k = divmod(c, 4)
        heads = [SLOT_BASE[s] + k for s in range(HPC)]
        aq = np.zeros((HPC, AUG, S), np.float32)
        for s, h in enumerate(heads):
            aq[s, 0, :] = slopes_bf[h]
            aq[s, 1, :] = slopes_bf[h]
            aq[s, 2, :] = -slopes_bf[h] * iv
        cols = np.concatenate(
            [np.arange(h * HWID, (h + 1) * HWID) for h in heads])
        in_maps.append({
            "xq": qTs[b], "xk": kTs[b], "xv": vTs[b],
            "wq": asc(WqT[:, cols]), "wk": asc(WkT[:, cols]),
            "wv": asc(WvT[:, cols]),
            "augq": aq.astype(BF16NP), "augk": augk_np,
            "tri": tri_np,
        })

    res = run_bass_kernel_spmd(nc, in_maps, core_ids=list(range(8)))
    outp = np.empty((B, S, D), np.float32)
    for c in range(8):
        b, k = divmod(c, 4)
        for s in range(HPC):
            h = SLOT_BASE[s] + k
            outp[b, :, h * HWID:(h + 1) * HWID] = \
                res.results[c]["out"][:, s * HWID:(s + 1) * HWID]
    return outp


# revision 68
# speedup vs baseline: 1.1730x; 1.0050x over previous
"""MHSA (dense transformer, ALiBi + causal) TRN2 Bass kernel, 8-core SPMD.

v4 design:
- Sharding: batch (2) x head-quartile -> 8 cores, heads REBALANCED so every
  core gets one head from each ALiBi-slope quartile: core c (b=c//4, k=c%4)
  computes heads [12+k, 8+k, 4+k, 0+k] (0-indexed) of batch b. Slot s on all
  cores then shares one causal band -> SPMD-friendly block skipping.
- Banded causal attention: ALiBi slope*(i-j) >~ 32 => weight < e^-32,
  skipped structurally. Bands (in 128-blocks) per slot: [16, 16, 5, 2].
- All matmuls bf16 (1 cyc/row on PE at any width). ALiBi folded into 3
  bf16-exact aug contraction rows:
    Q~ = [Q; slope; slope; -slope*i],  K~ = [K; j_hi; j_lo; 1]
  with j_hi multiple of 256 and j_lo in [0,256) - both exact in bf16; the
  -slope*i row is a per-row shift that cancels in softmax.
- Projection phase (x/w bf16, PSUM f32): Q^T/K^T copied to bf16 SBUF slot
  tiles by DVE, V by DVE into a [j, slot, 65] bf16 tile with a ones column
  (column 64 of the AV output becomes the softmax denominator).
- Attention interleaved with projections by i-window: window t = i in
  [512t, 512t+512), its strip groups round-robined with the projection
  chains of superblock t+1 so ACT exp overlaps PE projection matmuls.
- Scores S^T[j,i] per (slot, J): strips grouped into <=1024-col PSUM tiles,
  ONE exp (ACT) per group -> bf16 P^T; diagonal 128-strips masked on Pool
  (gpsimd) with a 0/1 triangle.
- AV: out[i(128), 65] accumulated over J into a [128, 4, 80] PSUM tile.
  IMPORTANT: PSUM accumulation groups are tracked per 2KB bank - only ONE
  chain may be open per bank at a time (interleaved starts lazily re-zero
  the bank). So all AV chains of a window are emitted at window END, one
  i-block at a time, each chain fully closed before the next starts.
- Epilogue = batched reciprocal + tensor_scalar multiplies (DVE) into an
  SBUF staging tile, DMA'd out per slot-pair (512B elements).
"""

import numpy as np
import ml_dtypes

import concourse.bass as bass
import concourse.mybir as mybir
import concourse.tile as tile
from concourse import bacc
from concourse.bass_utils import run_bass_kernel_spmd

P = 128
S = 2048
D = 1024
H = 16
HWID = 64
HPC = 4           # head slots per core
CW = HPC * HWID   # 256
NKC = D // P      # 8 contraction chunks
NTSB = 4          # t super-blocks (projection + attention i-windows)
TSB = S // NTSB   # 512
NJ = S // P       # 16
AUG = 3
QROWS = HWID + AUG  # 67
VSTR = 72           # V~ sub-block stride: 64 data + aligned ones lane
VONE = 64           # ones column (softmax denominator), first col of the lane
BANDS = [16, 16, 5, 2]      # causal band per slot, in 128-blocks
SLOT_BASE = [12, 8, 4, 0]   # head (0-indexed) = SLOT_BASE[s] + (core % 4)

F32 = mybir.dt.float32
BF16 = mybir.dt.bfloat16

EXP_BIAS = -12.0
BF16NP = ml_dtypes.bfloat16


def window_strips(s, t):
    """Strips (s, J, i0, i1) of window t for slot s (banded causal)."""
    B = BANDS[s]
    res = []
    for J in range(max(0, 4 * t - B + 1), 4 * t + 4):
        i0 = max(TSB * t, P * J)
        i1 = min(TSB * t + TSB, P * (J + B), S)
        if i1 > i0:
            res.append((s, J, i0, i1))
    return res


def group_strips(strips_w, cap=1024):
    groups, cur, w = [], [], 0
    for (s, J, i0, i1) in strips_w:
        if w + (i1 - i0) > cap and cur:
            groups.append(cur)
            cur, w = [], 0
        cur.append((s, J, i0, i1))
        w += i1 - i0
    if cur:
        groups.append(cur)
    return groups


def build_kernel():
    nc = bacc.Bacc("TRN2")

    xq = nc.dram_tensor("xq", [D, S], BF16, kind="ExternalInput")
    xk = nc.dram_tensor("xk", [D, S], BF16, kind="ExternalInput")
    xv = nc.dram_tensor("xv", [D, S], BF16, kind="ExternalInput")
    wq = nc.dram_tensor("wq", [D, CW], BF16, kind="ExternalInput")
    wk = nc.dram_tensor("wk", [D, CW], BF16, kind="ExternalInput")
    wv = nc.dram_tensor("wv", [D, CW], BF16, kind="ExternalInput")
    augq = nc.dram_tensor("augq", [HPC, AUG, S], BF16, kind="ExternalInput")
    augk = nc.dram_tensor("augk", [AUG, S], BF16, kind="ExternalInput")
    tri = nc.dram_tensor("tri", [P, P], BF16, kind="ExternalInput")
    out = nc.dram_tensor("out", [S, CW], F32, kind="ExternalOutput")

    x_drams = [xq, xk, xv]
    w_drams = [wq, wk, wv]

    with tile.TileContext(nc) as tc:
        with (
            tc.tile_pool(name="cn", bufs=1) as cnp,
            tc.tile_pool(name="wp", bufs=1) as wp,
            tc.tile_pool(name="qk", bufs=1) as qkp,
            tc.tile_pool(name="vv", bufs=1) as vvp,
            tc.tile_pool(name="xp", bufs=3) as xp,
            tc.tile_pool(name="pt", bufs=16) as ptp,
            tc.tile_pool(name="rc", bufs=3) as rcp,
            tc.tile_pool(name="ob", bufs=1) as obp,
            tc.tile_pool(name="pq", bufs=2, space="PSUM") as pqp,
            tc.tile_pool(name="sc", bufs=2, space="PSUM") as scp,
            tc.tile_pool(name="av", bufs=2, space="PSUM") as avp,
        ):
            # ---- startup DMAs in critical-path order: (wv, xv0), (wq, xq0),
            # slot-0/1 augs, (wk, xk0), slot-2/3 augs — V/Q chains and the
            # first windows (slots 0/1) start as soon as their data lands ----
            def emit_xdma_one(xt, t, xi):
                nc.sync.dma_start(
                    xt[:, :, xi, :],
                    x_drams[xi].rearrange("(ko p) t -> p ko t", p=P)[
                        :, :, t * TSB:(t + 1) * TSB],
                )

            def emit_xdma(t):
                xt = xp.tile([P, NKC, 3, TSB], BF16, tag="x", name=f"x{t}")
                for xi in range(3):
                    emit_xdma_one(xt, t, xi)
                return xt

            w_tiles = [None, None, None]
            xt0 = xp.tile([P, NKC, 3, TSB], BF16, tag="x", name="x0")

            def emit_wdma(i):
                wt = wp.tile([P, NKC, CW], BF16, tag=f"w{i}", name=f"w{i}")
                nc.sync.dma_start(
                    wt[:], w_drams[i].rearrange("(ko p) c -> p ko c", p=P))
                w_tiles[i] = wt

            emit_wdma(2)
            # xv in halves: V chains u=0,1 need only cols 0:256
            nc.sync.dma_start(
                xt0[:, :, 2, 0:256],
                x_drams[2].rearrange("(ko p) t -> p ko t", p=P)[:, :, 0:256])
            nc.sync.dma_start(
                xt0[:, :, 2, 256:512],
                x_drams[2].rearrange("(ko p) t -> p ko t", p=P)[:, :, 256:512])
            emit_wdma(0)
            nc.sync.dma_start(
                xt0[:, :, 0, 0:256],
                x_drams[0].rearrange("(ko p) t -> p ko t", p=P)[:, :, 0:256])
            nc.sync.dma_start(
                xt0[:, :, 0, 256:512],
                x_drams[0].rearrange("(ko p) t -> p ko t", p=P)[:, :, 256:512])

            # ---- constants ----
            tri_t = cnp.tile([P, P], BF16, tag="tri", name="tri_t")
            nc.sync.dma_start(tri_t[:], tri[:])
            ebias = cnp.tile([P, 1], F32, tag="ebias", name="ebias")
            nc.gpsimd.memset(ebias[:], EXP_BIAS)

            # ---- per-slot Q~^T / K~^T [67, S] bf16; V~ [128, NJ, HPC, 65] ----
            qs = [qkp.tile([QROWS, S], BF16, tag=f"q{s}", name=f"q{s}")
                  for s in range(HPC)]
            ks = [qkp.tile([QROWS, S], BF16, tag=f"k{s}", name=f"k{s}")
                  for s in range(HPC)]
            # V~ sub-block = 72 cols: V data 0:64, then a 16-byte aligned
            # constant lane [64:72) holding the softmax-denominator ones at
            # col VONE. Padding keeps any >=4-byte RMW write granule away
            # from the bf16 V data (same-engine memsets, ordered before the
            # DVE V copies).
            v_t = vvp.tile([P, NJ, HPC, VSTR], BF16, tag="v", name="v_t")
            nc.vector.memset(v_t[:, :, :, HWID:VSTR], 0.0)
            nc.vector.memset(v_t[:, :, :, VONE:VONE + 1], 1.0)

            # slot-0/1 aug rows early (first windows), then (wk, xk0),
            # then slot-2/3 augs
            for s in range(2):
                nc.sync.dma_start(qs[s][HWID:QROWS, :], augq[s])
                nc.sync.dma_start(ks[s][HWID:QROWS, :], augk[:])
            emit_wdma(1)
            emit_xdma_one(xt0, 0, 1)
            for s in range(2, HPC):
                nc.sync.dma_start(qs[s][HWID:QROWS, :], augq[s])
                nc.sync.dma_start(ks[s][HWID:QROWS, :], augk[:])

            # ---- output staging [P, NJ, CW] f32 ----
            out_sb = obp.tile([P, NJ, CW], F32, tag="ob", name="out_sb")

            # ---------- emission helpers ----------
            def cc_chain_units(t, xt, cc):
                return [("qk", t, xt, pi, cc) for pi in range(2)]

            def v_chain_units(t, xt):
                return [("v", t, xt, u) for u in range(TSB // P)]

            def emit_chain(unit):
                kind = unit[0]
                if kind == "qk":
                    _, t, xt, pi, cc = unit
                    dsts = qs if pi == 0 else ks
                    ps = pqp.tile([P, TSB], F32, tag="pq",
                                  name=f"pq{t}_{pi}_{cc}")
                    # t0 Q cc0: two sequential half-chains so the first
                    # starts after the half x-DMA lands
                    halves = ([(0, 256), (256, TSB)]
                              if (t == 0 and pi == 0 and cc == 0)
                              else [(0, TSB)])
                    for (a, b) in halves:
                        for kk in range(NKC):
                            nc.tensor.matmul(
                                ps[:, a:b],
                                lhsT=w_tiles[pi][:, kk, cc * P:(cc + 1) * P],
                                rhs=xt[:, kk, pi, a:b],
                                start=(kk == 0),
                                stop=(kk == NKC - 1),
                            )
                    nc.vector.tensor_copy(
                        dsts[2 * cc][0:HWID, t * TSB:(t + 1) * TSB],
                        ps[0:HWID, :],
                    )
                    nc.vector.tensor_copy(
                        dsts[2 * cc + 1][0:HWID, t * TSB:(t + 1) * TSB],
                        ps[HWID:P, :],
                    )
                else:
                    _, t, xt, u = unit
                    tt = t * (TSB // P) + u
                    ps = pqp.tile([P, TSB], F32, tag="pq", name=f"pv{t}_{u}")
                    for kk in range(NKC):
                        nc.tensor.matmul(
                            ps[:, 0:CW],
                            lhsT=xt[:, kk, 2, u * P:(u + 1) * P],
                            rhs=w_tiles[2][:, kk, :],
                            start=(kk == 0),
                            stop=(kk == NKC - 1),
                        )
                    nc.vector.tensor_copy(
                        v_t[:, tt, :, 0:HWID],
                        ps[:, 0:CW].rearrange("p (h w) -> p h w", h=HPC),
                    )

            def emit_scores_exp(t, g, gi):
                width = sum(i1 - i0 for (_, _, i0, i1) in g)
                sc = scp.tile([P, 1024], F32, tag="sc", name=f"sc{t}_{gi}")
                o = 0
                offs = []
                for (s, J, i0, i1) in g:
                    W = i1 - i0
                    a = 0
                    while a < W:  # split at psum bank boundaries (512 cols)
                        b = min(W, a + 512 - (o + a) % 512)
                        nc.tensor.matmul(
                            sc[:, o + a:o + b],
                            lhsT=ks[s][0:QROWS, P * J:P * (J + 1)],
                            rhs=qs[s][0:QROWS, i0 + a:i0 + b],
                            start=True,
                            stop=True,
                        )
                        a = b
                    offs.append((s, J, i0, i1, o))
                    o += W
                pt = ptp.tile([P, 1024], BF16, tag="pt", name=f"pt{t}_{gi}")
                nc.scalar.activation(
                    pt[:, 0:width], sc[:, 0:width],
                    mybir.ActivationFunctionType.Exp,
                    bias=ebias[:], scale=1.0,
                )
                for (s, J, i0, i1, off) in offs:
                    if i0 == P * J:
                        # mask the diagonal 128-strip (keep j <= i), on Pool
                        nc.gpsimd.tensor_mul(
                            pt[:, off:off + P], pt[:, off:off + P], tri_t[:]
                        )
                return pt, offs

            def emit_epilogue(s, t, acc):
                rec = rcp.tile([P, 4, 1], F32, tag="rc", name=f"rc{s}_{t}")
                nc.vector.reciprocal(rec[:], acc[:, :, VONE:VONE + 1])
                def pair_out(a0, a1):
                    nc.sync.dma_start(
                        out.rearrange("(a p) c -> p a c", p=P)[
                            :, a0:a1, (s - 1) * HWID:(s + 1) * HWID],
                        out_sb[:, a0:a1, (s - 1) * HWID:(s + 1) * HWID],
                    )

                last = s % 2 == 1 and t == NTSB - 1
                for r in range(4):
                    nc.vector.tensor_scalar_mul(
                        out_sb[:, 4 * t + r, s * HWID:(s + 1) * HWID],
                        acc[:, r, 0:HWID],
                        rec[:, r, :],
                    )
                    if last and r in (1, 3):
                        # tail rows stream out as their norms land
                        pair_out(*((12, 14) if r == 1 else (14, 16)))
                if s % 2 == 1 and t in (1, 2):
                    # finished row-blocks stream out per block
                    a0, a1 = (0, 8) if t == 1 else (8, 12)
                    pair_out(a0, a1)

            # pending completed windows: list of [s, t, jmap, age]
            pends = []
            vq = []       # current block's deferred V chains (unemitted)
            cur_t = [0]

            def flush_window():
                if not pends:
                    return
                if pends[0][1] == cur_t[0]:
                    # flushing a window of the CURRENT block: its AV reads
                    # v_t of this superblock - drain the deferred V chains
                    # first (they must precede the read in program order)
                    while vq:
                        emit_chain(vq.pop(0))
                s, t, jmap, _ = pends.pop(0)
                B = BANDS[s]
                acc = avp.tile([P, 4, 80], F32, tag="acc", name=f"acc{s}_{t}")
                # one fully-closed accumulation chain per i-block (PSUM bank
                # allows only one open chain at a time)
                for r in range(4):
                    i_blk = 4 * t + r
                    jst = max(0, i_blk - B + 1)
                    for J in range(jst, i_blk + 1):
                        pt, off, i0 = jmap[J]
                        col = off + i_blk * P - i0
                        nc.tensor.matmul(
                            acc[:, r, 0:VONE + 1],
                            lhsT=pt[:, col:col + P],
                            rhs=v_t[:, J, s, 0:VONE + 1],
                            start=(J == jst),
                            stop=(J == i_blk),
                        )
                emit_epilogue(s, t, acc)

            # ---------- main schedule ----------
            # upfront: V(0) (xv lands first) then Q/K cc0(0) - just enough
            # for the first windows (slots 0/1). Everything else defers into
            # the attention blocks as PE filler, latest-needed last:
            # block t = [cc1(t): its s2/s3 scores] + [V(t): its AV flushes]
            #         + [cc0(t+1): next block's s0/s1 scores]
            for unit in v_chain_units(0, xt0) + cc_chain_units(0, xt0, 0):
                emit_chain(unit)

            xts = {0: xt0}

            for t in range(NTSB):
                cur_t[0] = t
                nv = 0
                if t > 0:
                    vq.extend(v_chain_units(t, xts[t]))
                    nv = len(vq)
                cc1q = cc_chain_units(t, xts[t], 1)
                ncc1 = len(cc1q)
                next_chains = []
                if t + 1 < NTSB:
                    xt_next = emit_xdma(t + 1)
                    xts[t + 1] = xt_next
                    next_chains = cc_chain_units(t + 1, xt_next, 0)
                all_strips = []
                last_of = {}
                for s in range(HPC):
                    ws = window_strips(s, t)
                    all_strips.extend(ws)
                    last_of[ws[-1]] = s
                if t == NTSB - 1:
                    # split the final slot's groups so the last serial exp
                    # in the tail is short
                    head = [st for st in all_strips if st[0] < 3]
                    tl3 = [st for st in all_strips if st[0] == 3]
                    groups = group_strips(head) + group_strips(tl3, cap=512)
                else:
                    groups = group_strips(all_strips)
                total_groups = len(groups)
                first_s2 = next((i for i, g in enumerate(groups)
                                 if any(st[0] >= 2 for st in g)),
                                total_groups)

                ci = 0
                jmaps = {s: {} for s in range(HPC)}
                for gi, g in enumerate(groups):
                    if cc1q and any(st[0] >= 2 for st in g):
                        # this group's scores read qs/ks of slots 2/3:
                        # their projection chains must precede it
                        while cc1q:
                            emit_chain(cc1q.pop(0))
                    pt, offs = emit_scores_exp(t, g, gi)
                    for (s, J, i0, i1, off) in offs:
                        jmaps[s][J] = (pt, off, i0)
                    for pw in pends:
                        pw[3] += 1
                    for strip in g:
                        if strip in last_of:
                            pends.append([last_of[strip], t, jmaps[strip[0]], 0])
                    # V chains spread over the first half of the block,
                    # cc1 before the first s2/s3 scores
                    while vq and nv - len(vq) < ((gi + 1) * nv * 2) \
                            // total_groups:
                        emit_chain(vq.pop(0))
                    while cc1q and ncc1 - len(cc1q) < \
                            ((gi + 1) * ncc1) // max(1, first_s2):
                        emit_chain(cc1q.pop(0))
                    while (ci < len(next_chains)
                           and ci < (max(0, 2 * (gi + 1) - total_groups)
                                     * len(next_chains)) // total_groups):
                        emit_chain(next_chains[ci])
                        ci += 1
                    while pends and (pends[0][3] >= 10 or len(pends) > 4):
                        flush_window()
                while vq:
                    emit_chain(vq.pop(0))
                while cc1q:
                    emit_chain(cc1q.pop(0))
                while ci < len(next_chains):
                    emit_chain(next_chains[ci])
                    ci += 1
            while pends:
                flush_window()

    nc.compile()
    return nc


_NC = None


def _get_nc():
    global _NC
    if _NC is None:
        _NC = build_kernel()
    return _NC


def kernel(queries, keys, values, mask, Wq, Wk, Wv):
    B = queries.shape[0]
    asc = np.ascontiguousarray
    scale = 1.0 / np.sqrt(HWID)

    WqT = asc((np.asarray(Wq).T * scale).astype(np.float32)).astype(BF16NP)
    WkT = asc(np.asarray(Wk).T.astype(np.float32)).astype(BF16NP)
    WvT = asc(np.asarray(Wv).T.astype(np.float32)).astype(BF16NP)
    qTs = [asc(np.asarray(queries[b]).T.astype(np.float32)).astype(BF16NP)
           for b in range(B)]
    kTs = [asc(np.asarray(keys[b]).T.astype(np.float32)).astype(BF16NP)
           for b in range(B)]
    vTs = [asc(np.asarray(values[b]).T.astype(np.float32)).astype(BF16NP)
           for b in range(B)]

    slopes = (2.0 ** (-np.arange(1, H + 1) * (8.0 / H))).astype(np.float32)
    slopes_bf = slopes.astype(BF16NP).astype(np.float32)
    iv = np.arange(S, dtype=np.float32)
    j_hi = (np.arange(S) // 256 * 256).astype(np.float32)
    j_lo = (np.arange(S) % 256).astype(np.float32)
    augk_np = np.stack([j_hi, j_lo, np.ones(S, np.float32)]).astype(BF16NP)
    tri_np = np.asarray(
        np.arange(P)[:, None] <= np.arange(P)[None, :], dtype=np.float32
    ).astype(BF16NP)  # keep j<=i: rows p (j), cols u (i)

    nc = _get_nc()
    in_maps = []
    for c in range(8):
        b, k = divmod(c, 4)
        heads = [SLOT_BASE[s] + k for s in range(HPC)]
        aq = np.zeros((HPC, AUG, S), np.float32)
        for s, h in enumerate(heads):
            aq[s, 0, :] = slopes_bf[h]
            aq[s, 1, :] = slopes_bf[h]
            aq[s, 2, :] = -slopes_bf[h] * iv
        cols = np.concatenate(
            [np.arange(h * HWID, (h + 1) * HWID) for h in heads])
        in_maps.append({
            "xq": qTs[b], "xk": kTs[b], "xv": vTs[b],
            "wq": asc(WqT[:, cols]), "wk": asc(WkT[:, cols]),
            "wv": asc(WvT[:, cols]),
            "augq": aq.astype(BF16NP), "augk": augk_np,
            "tri": tri_np,
        })

    res = run_bass_kernel_spmd(nc, in_maps, core_ids=list(range(8)))
    outp = np.empty((B, S, D), np.float32)
    for c in range(8):
        b, k = divmod(c, 4)
        for s in range(HPC):
            h = SLOT_BASE[s] + k
            outp[b, :, h * HWID:(h + 1) * HWID] = \
                res.results[c]["out"][:, s * HWID:(s + 1) * HWID]
    return outp
